# revision 8
# baseline (speedup 1.0000x reference)
"""Bass/Tile TRN2 kernel for nn_LAN_4320737100678 (dense transformer block).

Data-parallel over the batch axis across 8 NeuronCores (4 batches/core).
All activations are kept feature-major ([E, L] per batch) so that every
BatchNorm reduction and the softmax run along the free axis. The five
BatchNorm moment sets are globalized with four tiny in-kernel AllReduces
(BN2+BN3 share one round).

v3 (perf): all matmuls/transposes run in bf16 (1 cycle/row on the PE vs
4 for fp32 — the fp32 baseline was 100% TensorMatrix-bound). Engine
split keeps every engine under the PE's ~130us: the PSUM epilogue is one
fused DVE tensor_tensor_reduce (bias-add + sum accumulate), sum-of-
squares runs as batched per-chunk Square+accum on the Scalar engine,
ELU is exp (Scalar) + affine (DVE) + min/sub (Pool, SBUF-only) + max
(DVE), and the BN affine rsqrt is exp(-0.5*ln(var+eps)) so the Scalar
engine never leaves one activation-table set (the baseline burned 20us
on ACT_TABLE_LOAD thrash from Sqrt). The BN4 affine folds into the
softmax exp (bias cancels; scale rides the activation scale operand)
with one chunk-wide stabilizer max. Matmuls are emitted chunk-major so
the per-chunk stats passes overlap the matmul phase; l-transposes and
their PSUM drains fill the BN2/3 allreduce gap. Inputs load as a
handful of large batched DMAs.
"""

import os
import sys

sys.path.insert(0, "/opt/trn_rl_repo")

import numpy as np
import ml_dtypes

import concourse.bass as bass
import concourse.tile as tile
from concourse import mybir
from concourse.bass_utils import run_bass_kernel_spmd
from concourse.masks import make_identity

N_CORES = 8
B, L, E, W = 32, 512, 512, 5
S = W // 2
P = 128
KC = E // P            # feature chunks of 128
WK = W * KC            # stage-1 contraction tiles
B_LOC = B // N_CORES   # batches per core
EPS = 1e-3
F32 = mybir.dt.float32
BF16 = mybir.dt.bfloat16
AF = mybir.ActivationFunctionType
ALU = mybir.AluOpType
AX = mybir.AxisListType

# gpack column base offsets (each vector packed as [P, KC])
_G1, _B1, _G2, _B2, _G3, _B3, _G4, _B4, _G5, _B5 = (i * KC for i in range(10))

_MAX_CTRL_WAITS = 1


def _split_waits(nc, max_waits=_MAX_CTRL_WAITS):
    """walrus in this container encodes at most one sync-wait slot per
    instruction. Hoist extra waits onto same-engine NOPs inserted right
    before the owning instruction (same engine => executes first)."""
    for fn in nc.m.functions:
        for bb in fn.blocks:
            rebuilt = []
            changed = False
            for ins in bb.instructions:
                si = ins.sync_info
                if si is not None and len(si.on_wait) > max_waits:
                    waits = list(si.on_wait)
                    rest = waits[max_waits:]
                    for j in range(0, len(rest), max_waits):
                        nop = mybir.InstNoOp(
                            name=f"{ins.name}_wsplit{j}",
                            engine=ins.engine,
                            bass_nofuse=True,
                            sync_info=mybir.SyncInfo(
                                on_wait=rest[j : j + max_waits], on_update=[]
                            ),
                        )
                        rebuilt.append(nop)
                    ins.sync_info = mybir.SyncInfo(
                        on_wait=waits[:max_waits], on_update=list(si.on_update)
                    )
                    changed = True
                rebuilt.append(ins)
            if changed:
                bb.instructions = rebuilt


_CACHE = {}


def _build():
    if "nc" in _CACHE:
        return _CACHE["nc"]
    nc = bass.Bass("TRN2", target_bir_lowering=False, debug=False, num_devices=N_CORES)

    m1t_d = nc.dram_tensor("m1t", [B_LOC, E, L], BF16, kind="ExternalInput")
    f_d = nc.dram_tensor("f", [W * E, E], BF16, kind="ExternalInput")
    wq_d = nc.dram_tensor("wq", [E, E], BF16, kind="ExternalInput")
    wk_d = nc.dram_tensor("wk", [E, E], BF16, kind="ExternalInput")
    qbt_d = nc.dram_tensor("qbt", [E, L], F32, kind="ExternalInput")
    kbt_d = nc.dram_tensor("kbt", [E, L], F32, kind="ExternalInput")
    wbt_d = nc.dram_tensor("wbt", [L, L], F32, kind="ExternalInput")
    gp_d = nc.dram_tensor("gpack", [P, 10 * KC], F32, kind="ExternalInput")
    out_d = nc.dram_tensor("outt", [B_LOC, E, L], BF16, kind="ExternalOutput")

    groups = [list(range(N_CORES))]
    NBL = float(B * L)  # total elements per feature for BN moments

    from contextlib import ExitStack

    with tile.TileContext(nc) as tc:
        with (
            tc.tile_pool(name="const", bufs=1) as const,
            tc.tile_pool(name="bias", bufs=1) as biasp,
            tc.tile_pool(name="aff", bufs=12) as affp,
            tc.tile_pool(name="acc", bufs=24) as accp,
            tc.tile_pool(name="packs", bufs=8) as packp,
            tc.tile_pool(name="scr", bufs=16) as scr,
            tc.tile_pool(name="col", bufs=16) as colp,
            tc.tile_pool(name="elu", bufs=8) as elup,
            tc.tile_pool(name="junk", bufs=2) as junkp,
            tc.tile_pool(name="psum", bufs=4, space="PSUM") as psum,
            tc.tile_pool(name="psumT", bufs=4, space="PSUM") as psumT,
            tc.tile_pool(name="dram", bufs=8, space="DRAM") as dram,
        ):
            es_l = ExitStack()
            wtp = es_l.enter_context(tc.tile_pool(name="wT", bufs=KC))
            lp = es_l.enter_context(tc.tile_pool(name="l", bufs=KC))
            lsp = es_l.enter_context(tc.tile_pool(name="lstd", bufs=B_LOC * KC))
            gp = const.tile([P, 10 * KC], F32, tag="gp")
            nc.sync.dma_start(out=gp[:], in_=gp_d[:])
            ident = const.tile([P, P], BF16, tag="ident")
            make_identity(nc, ident[:])
            epst = const.tile([P, 1], F32, tag="eps")
            nc.vector.memset(epst[:], EPS)

            # Batched parameter loads (one DMA each).
            qbt_sb = biasp.tile([P, KC, L], F32, tag="qbt")
            nc.sync.dma_start(
                out=qbt_sb[:], in_=qbt_d[:].rearrange("(kc p) l -> p kc l", p=P)
            )
            kbt_sb = biasp.tile([P, KC, L], F32, tag="kbt")
            nc.sync.dma_start(
                out=kbt_sb[:], in_=kbt_d[:].rearrange("(kc p) l -> p kc l", p=P)
            )
            wbt_sb = biasp.tile([P, KC, L], F32, tag="wbt")
            nc.sync.dma_start(
                out=wbt_sb[:], in_=wbt_d[:].rearrange("(kc p) l -> p kc l", p=P)
            )

            def allreduce(pack, width):
                """pack: [P, width] f32 of per-core moment sums (pre-scaled).
                Returns the cross-core sum."""
                cc_in = dram.tile([P, width], F32, tag="cc", name="cc_in")
                cc_out = dram.tile([P, width], F32, tag="cc", name="cc_out")
                nc.gpsimd.dma_start(out=cc_in[:], in_=pack[:])
                nc.gpsimd.collective_compute(
                    "AllReduce",
                    ALU.add,
                    replica_groups=groups,
                    ins=[cc_in.opt()],
                    outs=[cc_out.opt()],
                )
                g = packp.tile([P, width], F32, tag="g", name="g")
                nc.gpsimd.dma_start(out=g[:], in_=cc_out[:])
                return g

            def make_pack(az, azq, name):
                """az: [P, KC, B_LOC] f32 per-tile sums; azq: [P, KC] f32
                per-chunk sum of squares. Returns [P, KC, 2] pack of
                (mean, E[x^2]) scaled by 1/(B*L) ready for allreduce-add."""
                pack = packp.tile([P, KC, 2], F32, tag="pk", name=name)
                nc.vector.tensor_reduce(
                    out=pack[:, :, 0], in_=az[:], axis=AX.X, op=ALU.add
                )
                nc.vector.tensor_copy(pack[:, :, 1], azq[:])
                nc.vector.tensor_scalar_mul(pack[:], pack[:], 1.0 / NBL)
                return pack

            def affines(g, gcol, bcol, want_bias=True):
                """From allreduced [P, KC*2] (mean, E[x^2]) compute
                sc[P, KC] = gamma*rsqrt(var+eps), bi[P, KC] = beta - mean*sc.
                rsqrt as exp(-0.5*ln(var+eps)) keeps the Scalar engine inside
                one activation-table set."""
                gv = g.rearrange("p (c two) -> p c two", two=2)
                mean = gv[:, :, 0]
                ex2 = gv[:, :, 1]
                sq = scr.tile([P, KC], F32, tag="scr", name="sq2")
                nc.vector.tensor_mul(sq[:], mean, mean)
                var = scr.tile([P, KC], F32, tag="scr", name="var")
                nc.vector.tensor_sub(var[:], ex2, sq[:])
                lnv = scr.tile([P, KC], F32, tag="scr", name="lnv")
                nc.scalar.activation(out=lnv[:], in_=var[:], func=AF.Ln, bias=epst[:])
                nc.vector.tensor_scalar_mul(lnv[:], lnv[:], -0.5)
                rsq = scr.tile([P, KC], F32, tag="scr", name="rsq")
                nc.scalar.activation(out=rsq[:], in_=lnv[:], func=AF.Exp)
                sc = affp.tile([P, KC], F32, tag="aff", name="sc")
                nc.vector.tensor_mul(sc[:], rsq[:], gp[:, gcol : gcol + KC])
                if not want_bias:
                    return sc, None
                tb = scr.tile([P, KC], F32, tag="scr", name="tb")
                nc.vector.tensor_mul(tb[:], mean, sc[:])
                bi = affp.tile([P, KC], F32, tag="aff", name="bi")
                nc.vector.tensor_sub(bi[:], gp[:, bcol : bcol + KC], tb[:])
                return sc, bi

            def sumsq_chunk(z_chunk, azq, mc):
                """Scalar engine: one batched Square+accum pass over a whole
                [P, B_LOC, L] chunk -> azq[:, mc] = sum(z^2)."""
                junk = junkp.tile([P, B_LOC, L], BF16, tag="junk", name="junk")
                nc.scalar.activation(
                    out=junk[:], in_=z_chunk[:], func=AF.Square,
                    accum_out=azq[:, mc : mc + 1],
                )

            def elu_tile(zt, sc, bi):
                """zt <- elu(y), y = zt*sc + bi, via max(y, min(exp(y),1)-1).
                Scalar: exp.  DVE: y affine + final max.  Pool: min/sub."""
                e = elup.tile([P, L], BF16, tag="elu_e", name="elu_e")
                y = elup.tile([P, L], BF16, tag="elu_y", name="elu_y")
                nc.scalar.activation(out=e[:], in_=zt, func=AF.Exp, bias=bi, scale=sc)
                nc.vector.tensor_scalar(
                    out=y[:], in0=zt, scalar1=sc, scalar2=bi,
                    op0=ALU.mult, op1=ALU.add,
                )
                nc.gpsimd.tensor_scalar(
                    out=e[:], in0=e[:], scalar1=1.0, scalar2=1.0,
                    op0=ALU.min, op1=ALU.subtract,
                )
                nc.vector.tensor_tensor(out=zt, in0=y[:], in1=e[:], op=ALU.max)

            # ---------------- Stage 1: z1 = unfold(m1) @ f + kb ----------------
            # l_all[mc]: [P, B_LOC, L] holds the full chunk across batches.
            l_all = [lp.tile([P, B_LOC, L], BF16, tag="l", name="l") for _ in range(KC)]
            az1 = accp.tile([P, KC, B_LOC], F32, tag="az", name="az1")
            azq1 = accp.tile([P, KC], F32, tag="azq", name="azq1")
            es_s1 = ExitStack()
            if True:
                fp = es_s1.enter_context(tc.tile_pool(name="f", bufs=1))
                mp = es_s1.enter_context(tc.tile_pool(name="m1", bufs=1))
                f_sb = fp.tile([P, WK, E], BF16, tag="f")
                nc.sync.dma_start(
                    out=f_sb[:], in_=f_d[:].rearrange("(i p) e -> p i e", p=P)
                )
                m1_sb = mp.tile([P, B_LOC, KC, L + 2 * S], BF16, tag="m1")
                nc.vector.memset(m1_sb[:, :, :, 0:S], 0.0)
                nc.vector.memset(m1_sb[:, :, :, S + L : 2 * S + L], 0.0)
                nc.sync.dma_start(
                    out=m1_sb[:, :, :, S : S + L],
                    in_=m1t_d[:].rearrange("b (kc p) l -> p b kc l", p=P),
                )

                # chunk-major so the per-chunk Square pass overlaps compute
                for mc in range(KC):
                    for b in range(B_LOC):
                        ps = psum.tile([P, L], F32, tag="ps", name="ps")
                        n = 0
                        for w in range(W):
                            for kc in range(KC):
                                nc.tensor.matmul(
                                    ps[:],
                                    f_sb[:, w * KC + kc, mc * P : (mc + 1) * P],
                                    m1_sb[:, b, kc, w : w + L],
                                    start=(n == 0),
                                    stop=(n == WK - 1),
                                )
                                n += 1
                        nc.vector.scalar_tensor_tensor(
                            out=l_all[mc][:, b, :], in0=ps[:], scalar=1.0,
                            in1=kbt_sb[:, mc, :], op0=ALU.mult, op1=ALU.add,
                            accum_out=az1[:, mc, b : b + 1],
                        )
                    sumsq_chunk(l_all[mc], azq1, mc)

            pack1 = make_pack(az1, azq1, "pack1")
            g1 = allreduce(pack1, KC * 2)
            sc1, bi1 = affines(g1, _G1, _B1)
            for b in range(B_LOC):
                for mc in range(KC):
                    elu_tile(
                        l_all[mc][:, b, :],
                        sc1[:, mc : mc + 1], bi1[:, mc : mc + 1],
                    )

            es_s1.close()

            # ------------- Stage 2/3: q2 = l@wq + qb, k2 = l@wk + kb -------------
            az2 = accp.tile([P, KC, B_LOC], F32, tag="az", name="az2")
            azq2 = accp.tile([P, KC], F32, tag="azq", name="azq2")
            az3 = accp.tile([P, KC, B_LOC], F32, tag="az", name="az3")
            azq3 = accp.tile([P, KC], F32, tag="azq", name="azq3")
            es_z = ExitStack()
            if True:
                z2p = es_z.enter_context(tc.tile_pool(name="z2", bufs=KC))
                z3p = es_z.enter_context(tc.tile_pool(name="z3", bufs=KC))
                wqkp = es_z.enter_context(tc.tile_pool(name="wqk", bufs=1))
                wq_sb = wqkp.tile([P, KC, E], BF16, tag="wq")
                nc.sync.dma_start(
                    out=wq_sb[:], in_=wq_d[:].rearrange("(kc p) e -> p kc e", p=P)
                )
                wk_sb = wqkp.tile([P, KC, E], BF16, tag="wk")
                nc.sync.dma_start(
                    out=wk_sb[:], in_=wk_d[:].rearrange("(kc p) e -> p kc e", p=P)
                )

                q2_all = [z2p.tile([P, B_LOC, L], BF16, tag="z2", name="z2") for _ in range(KC)]
                k2_all = [z3p.tile([P, B_LOC, L], BF16, tag="z3", name="z3") for _ in range(KC)]
                for mc in range(KC):
                    for b in range(B_LOC):
                        ps = psum.tile([P, L], F32, tag="ps", name="ps")
                        for kc in range(KC):
                            nc.tensor.matmul(
                                ps[:],
                                wq_sb[:, kc, mc * P : (mc + 1) * P],
                                l_all[kc][:, b, :],
                                start=(kc == 0),
                                stop=(kc == KC - 1),
                            )
                        nc.vector.scalar_tensor_tensor(
                            out=q2_all[mc][:, b, :], in0=ps[:], scalar=1.0,
                            in1=qbt_sb[:, mc, :], op0=ALU.mult, op1=ALU.add,
                            accum_out=az2[:, mc, b : b + 1],
                        )

                        ps = psum.tile([P, L], F32, tag="ps", name="ps")
                        for kc in range(KC):
                            nc.tensor.matmul(
                                ps[:],
                                wk_sb[:, kc, mc * P : (mc + 1) * P],
                                l_all[kc][:, b, :],
                                start=(kc == 0),
                                stop=(kc == KC - 1),
                            )
                        nc.vector.scalar_tensor_tensor(
                            out=k2_all[mc][:, b, :], in0=ps[:], scalar=1.0,
                            in1=kbt_sb[:, mc, :], op0=ALU.mult, op1=ALU.add,
                            accum_out=az3[:, mc, b : b + 1],
                        )
                    sumsq_chunk(q2_all[mc], azq2, mc)
                    sumsq_chunk(k2_all[mc], azq3, mc)

                pack23 = packp.tile([P, 2 * KC, 2], F32, tag="pk23", name="pack23")
                nc.vector.tensor_reduce(
                    out=pack23[:, 0:KC, 0], in_=az2[:], axis=AX.X, op=ALU.add
                )
                nc.vector.tensor_copy(pack23[:, 0:KC, 1], azq2[:])
                nc.vector.tensor_reduce(
                    out=pack23[:, KC : 2 * KC, 0], in_=az3[:], axis=AX.X, op=ALU.add
                )
                nc.vector.tensor_copy(pack23[:, KC : 2 * KC, 1], azq3[:])
                nc.vector.tensor_scalar_mul(pack23[:], pack23[:], 1.0 / NBL)
                g23 = allreduce(pack23, 4 * KC)

                # While the allreduce is in flight: transpose l on the PE
                # (stage 5 needs sequence-major l), drain via DVE copies.
                lstd_sb = {}
                for b in range(B_LOC):
                    for kc in range(KC):
                        pst = psumT.tile([P, E], BF16, tag="psT", name="psT")
                        for mc in range(KC):
                            nc.tensor.transpose(
                                pst[:, mc * P : (mc + 1) * P],
                                l_all[mc][:, b, kc * P : (kc + 1) * P],
                                ident[:],
                            )
                        lst = lsp.tile([P, E], BF16, tag="lstd", name="lstd")
                        nc.vector.tensor_copy(lst[:], pst[:])
                        lstd_sb[b, kc] = lst

                sc2, bi2 = affines(g23[:, 0 : 2 * KC], _G2, _B2)
                sc3, bi3 = affines(g23[:, 2 * KC : 4 * KC], _G3, _B3)

                for b in range(B_LOC):
                    for mc in range(KC):
                        elu_tile(
                            q2_all[mc][:, b, :],
                            sc2[:, mc : mc + 1], bi2[:, mc : mc + 1],
                        )
                        elu_tile(
                            k2_all[mc][:, b, :],
                            sc3[:, mc : mc + 1], bi3[:, mc : mc + 1],
                        )

                # ------------- Stage 4a: wT = (q2 @ k2^T)^T + wb^T -------------
                az4 = accp.tile([P, KC, B_LOC], F32, tag="az", name="az4")
                azq4 = accp.tile([P, KC], F32, tag="azq", name="azq4")
                wt_all = [wtp.tile([P, B_LOC, L], BF16, tag="wT", name="wT") for _ in range(KC)]
                wtmax = colp.tile([P, KC, B_LOC], F32, tag="mx", name="wtmax")
                for kc in range(KC):
                    for b in range(B_LOC):
                        ps = psum.tile([P, L], F32, tag="ps", name="ps")
                        for ec in range(KC):
                            nc.tensor.matmul(
                                ps[:],
                                k2_all[ec][:, b, kc * P : (kc + 1) * P],
                                q2_all[ec][:, b, :],
                                start=(ec == 0),
                                stop=(ec == KC - 1),
                            )
                        nc.vector.scalar_tensor_tensor(
                            out=wt_all[kc][:, b, :], in0=ps[:], scalar=1.0,
                            in1=wbt_sb[:, kc, :], op0=ALU.mult, op1=ALU.add,
                            accum_out=az4[:, kc, b : b + 1],
                        )
                    sumsq_chunk(wt_all[kc], azq4, kc)
                    # per-chunk max over (b, q) for the softmax stabilizer
                    nc.vector.tensor_reduce(
                        out=wtmax[:, kc, :], in_=wt_all[kc][:], axis=AX.X, op=ALU.max
                    )

            es_z.close()

            pack4 = make_pack(az4, azq4, "pack4")
            g4 = allreduce(pack4, KC * 2)
            # softmax(BN4(x)) over q: the BN4 bias cancels inside softmax, so
            # only the scale survives: softmax_q(sc4*x), stabilized with a
            # chunk-wide max folded into the exp bias.
            sc4, _ = affines(g4, _G4, _B4, want_bias=False)

            # ---------------- Stage 4b: softmax over q ----------------
            mxc = colp.tile([P, KC], F32, tag="mxc", name="mxc")
            nc.vector.tensor_reduce(out=mxc[:], in_=wtmax[:], axis=AX.X, op=ALU.max)
            mxs = colp.tile([P, KC], F32, tag="mxs", name="mxs")
            nc.vector.scalar_tensor_tensor(
                out=mxs[:], in0=mxc[:], scalar=-1.0, in1=sc4[:],
                op0=ALU.mult, op1=ALU.mult,
            )
            ssum = colp.tile([P, KC, B_LOC], F32, tag="ssum", name="ssum")
            rs = colp.tile([P, KC, B_LOC], F32, tag="rs", name="rs")
            for kc in range(KC):
                nc.scalar.activation(
                    out=wt_all[kc][:], in_=wt_all[kc][:], func=AF.Exp,
                    bias=mxs[:, kc : kc + 1], scale=sc4[:, kc : kc + 1],
                )
                nc.vector.tensor_reduce(
                    out=ssum[:, kc, :], in_=wt_all[kc][:], axis=AX.X, op=ALU.add
                )
            nc.vector.reciprocal(rs[:], ssum[:])
            for b in range(B_LOC):
                for kc in range(KC):
                    nc.gpsimd.tensor_scalar_mul(
                        wt_all[kc][:, b, :], wt_all[kc][:, b, :], rs[:, kc, b : b + 1]
                    )

            # ---------------- Stage 5: out = w @ l, BN5 + ELU ----------------
            az5 = accp.tile([P, KC, B_LOC], F32, tag="az", name="az5")
            azq5 = accp.tile([P, KC], F32, tag="azq", name="azq5")
            es_s5 = ExitStack()
            if True:
                outp = es_s5.enter_context(tc.tile_pool(name="out", bufs=KC))
                out_all = [outp.tile([P, B_LOC, L], BF16, tag="out", name="out") for _ in range(KC)]
                for mc in range(KC):
                    for b in range(B_LOC):
                        ps = psum.tile([P, L], F32, tag="ps", name="ps")
                        for kc in range(KC):
                            nc.tensor.matmul(
                                ps[:],
                                lstd_sb[b, kc][:, mc * P : (mc + 1) * P],
                                wt_all[kc][:, b, :],
                                start=(kc == 0),
                                stop=(kc == KC - 1),
                            )
                        nc.vector.tensor_scalar(
                            out=out_all[mc][:, b, :], in0=ps[:],
                            scalar1=1.0, scalar2=0.0, op0=ALU.mult, op1=ALU.add,
                            accum_out=az5[:, mc, b : b + 1],
                        )
                    sumsq_chunk(out_all[mc], azq5, mc)

                pack5 = make_pack(az5, azq5, "pack5")
                g5 = allreduce(pack5, KC * 2)
                sc5, bi5 = affines(g5, _G5, _B5)
                for mc in range(KC):
                    for b in range(B_LOC):
                        elu_tile(
                            out_all[mc][:, b, :],
                            sc5[:, mc : mc + 1], bi5[:, mc : mc + 1],
                        )
                    nc.sync.dma_start(
                        out=out_d[:, mc * P : (mc + 1) * P, :].rearrange(
                            "b p l -> p b l"
                        ),
                        in_=out_all[mc][:],
                    )

                es_s5.close()
                es_l.close()

    _split_waits(nc)
    _CACHE["nc"] = nc
    return nc


def _pack_affine(vecs):
    cols = []
    for v in vecs:
        cols.append(np.ascontiguousarray(np.asarray(v, np.float32).reshape(KC, P).T))
    return np.ascontiguousarray(np.concatenate(cols, axis=1))


def kernel(m1, f, wq, wk, qb, kb, wb, g1, b1, g2, b2, g3, b3, g4, b4, g5, b5):
    BF = ml_dtypes.bfloat16
    m1 = np.asarray(m1, np.float32)
    nc = _build()
    m1t = np.ascontiguousarray(m1.transpose(0, 2, 1)).astype(BF)
    f_h = np.ascontiguousarray(np.asarray(f, np.float32)).astype(BF)
    wq_h = np.ascontiguousarray(np.asarray(wq, np.float32)).astype(BF)
    wk_h = np.ascontiguousarray(np.asarray(wk, np.float32)).astype(BF)
    qbt = np.ascontiguousarray(np.asarray(qb, np.float32).T)
    kbt = np.ascontiguousarray(np.asarray(kb, np.float32).T)
    wbt = np.ascontiguousarray(np.asarray(wb, np.float32).T)
    gpack = _pack_affine([g1, b1, g2, b2, g3, b3, g4, b4, g5, b5])

    shared = {
        "f": f_h, "wq": wq_h, "wk": wk_h,
        "qbt": qbt, "kbt": kbt, "wbt": wbt, "gpack": gpack,
    }
    in_maps = [
        {"m1t": np.ascontiguousarray(m1t[i * B_LOC : (i + 1) * B_LOC]), **shared}
        for i in range(N_CORES)
    ]
    trace = os.environ.get("KERNEL_TRACE") == "1"
    res = run_bass_kernel_spmd(nc, in_maps, list(range(N_CORES)), trace=trace)
    _CACHE["last_results"] = res

    out = np.empty((B, L, E), np.float32)
    for i in range(N_CORES):
        out[i * B_LOC : (i + 1) * B_LOC] = (
            res.results[i]["outt"].astype(np.float32).transpose(0, 2, 1)
        )
    return out


# revision 10
# speedup vs baseline: 2.4093x; 2.4093x over previous
"""Bass/Tile TRN2 kernel for nn_LAN_4320737100678 (dense transformer block).

Data-parallel over the batch axis across 8 NeuronCores (4 batches/core).
All activations are kept feature-major ([E, L] per batch) so that every
BatchNorm reduction and the softmax run along the free axis. The five
BatchNorm moment sets are globalized with four tiny in-kernel AllReduces
(BN2+BN3 share one round).

v3 (perf): all matmuls/transposes run in bf16 (1 cycle/row on the PE vs
4 for fp32 — the fp32 baseline was 100% TensorMatrix-bound). Engine
split keeps every engine under the PE's ~130us: the PSUM epilogue is one
fused DVE tensor_tensor_reduce (bias-add + sum accumulate), sum-of-
squares runs as batched per-chunk Square+accum on the Scalar engine,
ELU is exp (Scalar) + affine (DVE) + min/sub (Pool, SBUF-only) + max
(DVE), and the BN affine rsqrt is exp(-0.5*ln(var+eps)) so the Scalar
engine never leaves one activation-table set (the baseline burned 20us
on ACT_TABLE_LOAD thrash from Sqrt). The BN4 affine folds into the
softmax exp (bias cancels; scale rides the activation scale operand)
with one chunk-wide stabilizer max. Matmuls are emitted chunk-major so
the per-chunk stats passes overlap the matmul phase; l-transposes and
their PSUM drains fill the BN2/3 allreduce gap. Inputs load as a
handful of large batched DMAs.
"""

import os
import sys

sys.path.insert(0, "/opt/trn_rl_repo")

import numpy as np
import ml_dtypes

import concourse.bass as bass
import concourse.tile as tile
from concourse import mybir
from concourse.bass_utils import run_bass_kernel_spmd
from concourse.masks import make_identity

N_CORES = 8
B, L, E, W = 32, 512, 512, 5
S = W // 2
P = 128
KC = E // P            # feature chunks of 128
WK = W * KC            # stage-1 contraction tiles
B_LOC = B // N_CORES   # batches per core
EPS = 1e-3
F32 = mybir.dt.float32
BF16 = mybir.dt.bfloat16
AF = mybir.ActivationFunctionType
ALU = mybir.AluOpType
AX = mybir.AxisListType

# gpack column base offsets (each vector packed as [P, KC])
_G1, _B1, _G2, _B2, _G3, _B3, _G4, _B4, _G5, _B5 = (i * KC for i in range(10))

_MAX_CTRL_WAITS = 1


def _split_waits(nc, max_waits=_MAX_CTRL_WAITS):
    """walrus in this container encodes at most one sync-wait slot per
    instruction. Hoist extra waits onto same-engine NOPs inserted right
    before the owning instruction (same engine => executes first)."""
    for fn in nc.m.functions:
        for bb in fn.blocks:
            rebuilt = []
            changed = False
            for ins in bb.instructions:
                si = ins.sync_info
                if si is not None and len(si.on_wait) > max_waits:
                    waits = list(si.on_wait)
                    rest = waits[max_waits:]
                    for j in range(0, len(rest), max_waits):
                        nop = mybir.InstNoOp(
                            name=f"{ins.name}_wsplit{j}",
                            engine=ins.engine,
                            bass_nofuse=True,
                            sync_info=mybir.SyncInfo(
                                on_wait=rest[j : j + max_waits], on_update=[]
                            ),
                        )
                        rebuilt.append(nop)
                    ins.sync_info = mybir.SyncInfo(
                        on_wait=waits[:max_waits], on_update=list(si.on_update)
                    )
                    changed = True
                rebuilt.append(ins)
            if changed:
                bb.instructions = rebuilt


_CACHE = {}


def _build():
    if "nc" in _CACHE:
        return _CACHE["nc"]
    nc = bass.Bass("TRN2", target_bir_lowering=False, debug=False, num_devices=N_CORES)

    m1t_d = nc.dram_tensor("m1t", [B_LOC, E, L], BF16, kind="ExternalInput")
    f_d = nc.dram_tensor("f", [W * E, E], BF16, kind="ExternalInput")
    wq_d = nc.dram_tensor("wq", [E, E], BF16, kind="ExternalInput")
    wk_d = nc.dram_tensor("wk", [E, E], BF16, kind="ExternalInput")
    qbt_d = nc.dram_tensor("qbt", [E, L], F32, kind="ExternalInput")
    kbt_d = nc.dram_tensor("kbt", [E, L], F32, kind="ExternalInput")
    wbt_d = nc.dram_tensor("wbt", [L, L], F32, kind="ExternalInput")
    gp_d = nc.dram_tensor("gpack", [P, 10 * KC], F32, kind="ExternalInput")
    out_d = nc.dram_tensor("outt", [B_LOC, E, L], BF16, kind="ExternalOutput")

    groups = [list(range(N_CORES))]
    NBL = float(B * L)  # total elements per feature for BN moments

    from contextlib import ExitStack

    with tile.TileContext(nc) as tc:
        with (
            tc.tile_pool(name="const", bufs=1) as const,
            tc.tile_pool(name="bias", bufs=1) as biasp,
            tc.tile_pool(name="aff", bufs=12) as affp,
            tc.tile_pool(name="acc", bufs=24) as accp,
            tc.tile_pool(name="packs", bufs=8) as packp,
            tc.tile_pool(name="scr", bufs=16) as scr,
            tc.tile_pool(name="col", bufs=16) as colp,
            tc.tile_pool(name="elu", bufs=8) as elup,
            tc.tile_pool(name="junk", bufs=2) as junkp,
            tc.tile_pool(name="psum", bufs=4, space="PSUM") as psum,
            tc.tile_pool(name="psumT", bufs=4, space="PSUM") as psumT,
            tc.tile_pool(name="dram", bufs=8, space="DRAM") as dram,
        ):
            es_l = ExitStack()
            wtp = es_l.enter_context(tc.tile_pool(name="wT", bufs=KC))
            lp = es_l.enter_context(tc.tile_pool(name="l", bufs=KC))
            lsp = es_l.enter_context(tc.tile_pool(name="lstd", bufs=B_LOC * KC))
            gp = const.tile([P, 10 * KC], F32, tag="gp")
            nc.sync.dma_start(out=gp[:], in_=gp_d[:])
            ident = const.tile([P, P], BF16, tag="ident")
            make_identity(nc, ident[:])
            epst = const.tile([P, 1], F32, tag="eps")
            nc.vector.memset(epst[:], EPS)

            # Batched parameter loads (one DMA each).
            qbt_sb = biasp.tile([P, KC, L], F32, tag="qbt")
            nc.sync.dma_start(
                out=qbt_sb[:], in_=qbt_d[:].rearrange("(kc p) l -> p kc l", p=P)
            )
            kbt_sb = biasp.tile([P, KC, L], F32, tag="kbt")
            nc.sync.dma_start(
                out=kbt_sb[:], in_=kbt_d[:].rearrange("(kc p) l -> p kc l", p=P)
            )
            wbt_sb = biasp.tile([P, KC, L], F32, tag="wbt")
            nc.sync.dma_start(
                out=wbt_sb[:], in_=wbt_d[:].rearrange("(kc p) l -> p kc l", p=P)
            )

            def allreduce(pack, width):
                """pack: [P, width] f32 of per-core moment sums (pre-scaled).
                Returns the cross-core sum."""
                cc_in = dram.tile([P, width], F32, tag="cc", name="cc_in")
                cc_out = dram.tile([P, width], F32, tag="cc", name="cc_out")
                nc.gpsimd.dma_start(out=cc_in[:], in_=pack[:])
                nc.gpsimd.collective_compute(
                    "AllReduce",
                    ALU.add,
                    replica_groups=groups,
                    ins=[cc_in.opt()],
                    outs=[cc_out.opt()],
                )
                g = packp.tile([P, width], F32, tag="g", name="g")
                nc.gpsimd.dma_start(out=g[:], in_=cc_out[:])
                return g

            def make_pack(az, azq, name):
                """az: [P, KC, B_LOC] f32 per-tile sums; azq: [P, KC] f32
                per-chunk sum of squares. Returns [P, KC, 2] pack of
                (mean, E[x^2]) scaled by 1/(B*L) ready for allreduce-add."""
                pack = packp.tile([P, KC, 2], F32, tag="pk", name=name)
                nc.vector.tensor_reduce(
                    out=pack[:, :, 0], in_=az[:], axis=AX.X, op=ALU.add
                )
                nc.vector.tensor_copy(pack[:, :, 1], azq[:])
                nc.vector.tensor_scalar_mul(pack[:], pack[:], 1.0 / NBL)
                return pack

            def affines(g, gcol, bcol, want_bias=True):
                """From allreduced [P, KC*2] (mean, E[x^2]) compute
                sc[P, KC] = gamma*rsqrt(var+eps), bi[P, KC] = beta - mean*sc.
                rsqrt as exp(-0.5*ln(var+eps)) keeps the Scalar engine inside
                one activation-table set."""
                gv = g.rearrange("p (c two) -> p c two", two=2)
                mean = gv[:, :, 0]
                ex2 = gv[:, :, 1]
                sq = scr.tile([P, KC], F32, tag="scr", name="sq2")
                nc.vector.tensor_mul(sq[:], mean, mean)
                var = scr.tile([P, KC], F32, tag="scr", name="var")
                nc.vector.tensor_sub(var[:], ex2, sq[:])
                lnv = scr.tile([P, KC], F32, tag="scr", name="lnv")
                nc.scalar.activation(out=lnv[:], in_=var[:], func=AF.Ln, bias=epst[:])
                nc.vector.tensor_scalar_mul(lnv[:], lnv[:], -0.5)
                rsq = scr.tile([P, KC], F32, tag="scr", name="rsq")
                nc.scalar.activation(out=rsq[:], in_=lnv[:], func=AF.Exp)
                sc = affp.tile([P, KC], F32, tag="aff", name="sc")
                nc.vector.tensor_mul(sc[:], rsq[:], gp[:, gcol : gcol + KC])
                if not want_bias:
                    return sc, None
                tb = scr.tile([P, KC], F32, tag="scr", name="tb")
                nc.vector.tensor_mul(tb[:], mean, sc[:])
                bi = affp.tile([P, KC], F32, tag="aff", name="bi")
                nc.vector.tensor_sub(bi[:], gp[:, bcol : bcol + KC], tb[:])
                return sc, bi

            def sumsq_chunk(z_chunk, azq, mc):
                """Scalar engine: one batched Square+accum pass over a whole
                [P, B_LOC, L] chunk -> azq[:, mc] = sum(z^2)."""
                junk = junkp.tile([P, B_LOC, L], BF16, tag="junk", name="junk")
                nc.scalar.activation(
                    out=junk[:], in_=z_chunk[:], func=AF.Square,
                    accum_out=azq[:, mc : mc + 1],
                )

            def elu_tile(zt, sc, bi):
                """zt <- elu(y), y = zt*sc + bi, via max(y, min(exp(y),1)-1).
                Scalar: exp.  DVE: y affine + final max.  Pool: min/sub."""
                e = elup.tile([P, L], BF16, tag="elu_e", name="elu_e")
                y = elup.tile([P, L], BF16, tag="elu_y", name="elu_y")
                nc.scalar.activation(out=e[:], in_=zt, func=AF.Exp, bias=bi, scale=sc)
                nc.vector.tensor_scalar(
                    out=y[:], in0=zt, scalar1=sc, scalar2=bi,
                    op0=ALU.mult, op1=ALU.add,
                )
                nc.vector.tensor_scalar(
                    out=e[:], in0=e[:], scalar1=1.0, scalar2=1.0,
                    op0=ALU.min, op1=ALU.subtract,
                )
                nc.vector.tensor_tensor(out=zt, in0=y[:], in1=e[:], op=ALU.max)

            # ---------------- Stage 1: z1 = unfold(m1) @ f + kb ----------------
            # l_all[mc]: [P, B_LOC, L] holds the full chunk across batches.
            l_all = [lp.tile([P, B_LOC, L], BF16, tag="l", name="l") for _ in range(KC)]
            az1 = accp.tile([P, KC, B_LOC], F32, tag="az", name="az1")
            azq1 = accp.tile([P, KC], F32, tag="azq", name="azq1")
            es_s1 = ExitStack()
            if True:
                fp = es_s1.enter_context(tc.tile_pool(name="f", bufs=1))
                mp = es_s1.enter_context(tc.tile_pool(name="m1", bufs=1))
                f_sb = fp.tile([P, WK, E], BF16, tag="f")
                nc.sync.dma_start(
                    out=f_sb[:], in_=f_d[:].rearrange("(i p) e -> p i e", p=P)
                )
                m1_sb = mp.tile([P, B_LOC, KC, L + 2 * S], BF16, tag="m1")
                nc.vector.memset(m1_sb[:, :, :, 0:S], 0.0)
                nc.vector.memset(m1_sb[:, :, :, S + L : 2 * S + L], 0.0)
                nc.sync.dma_start(
                    out=m1_sb[:, :, :, S : S + L],
                    in_=m1t_d[:].rearrange("b (kc p) l -> p b kc l", p=P),
                )

                # chunk-major so the per-chunk Square pass overlaps compute
                for mc in range(KC):
                    for b in range(B_LOC):
                        ps = psum.tile([P, L], F32, tag="ps", name="ps")
                        n = 0
                        for w in range(W):
                            for kc in range(KC):
                                nc.tensor.matmul(
                                    ps[:],
                                    f_sb[:, w * KC + kc, mc * P : (mc + 1) * P],
                                    m1_sb[:, b, kc, w : w + L],
                                    start=(n == 0),
                                    stop=(n == WK - 1),
                                )
                                n += 1
                        nc.vector.scalar_tensor_tensor(
                            out=l_all[mc][:, b, :], in0=ps[:], scalar=1.0,
                            in1=kbt_sb[:, mc, :], op0=ALU.mult, op1=ALU.add,
                            accum_out=az1[:, mc, b : b + 1],
                        )
                    sumsq_chunk(l_all[mc], azq1, mc)

            pack1 = make_pack(az1, azq1, "pack1")
            g1 = allreduce(pack1, KC * 2)
            sc1, bi1 = affines(g1, _G1, _B1)
            for b in range(B_LOC):
                for mc in range(KC):
                    elu_tile(
                        l_all[mc][:, b, :],
                        sc1[:, mc : mc + 1], bi1[:, mc : mc + 1],
                    )

            es_s1.close()

            # ------------- Stage 2/3: q2 = l@wq + qb, k2 = l@wk + kb -------------
            az2 = accp.tile([P, KC, B_LOC], F32, tag="az", name="az2")
            azq2 = accp.tile([P, KC], F32, tag="azq", name="azq2")
            az3 = accp.tile([P, KC, B_LOC], F32, tag="az", name="az3")
            azq3 = accp.tile([P, KC], F32, tag="azq", name="azq3")
            es_z = ExitStack()
            if True:
                z2p = es_z.enter_context(tc.tile_pool(name="z2", bufs=KC))
                z3p = es_z.enter_context(tc.tile_pool(name="z3", bufs=KC))
                wqkp = es_z.enter_context(tc.tile_pool(name="wqk", bufs=1))
                wq_sb = wqkp.tile([P, KC, E], BF16, tag="wq")
                nc.sync.dma_start(
                    out=wq_sb[:], in_=wq_d[:].rearrange("(kc p) e -> p kc e", p=P)
                )
                wk_sb = wqkp.tile([P, KC, E], BF16, tag="wk")
                nc.sync.dma_start(
                    out=wk_sb[:], in_=wk_d[:].rearrange("(kc p) e -> p kc e", p=P)
                )

                q2_all = [z2p.tile([P, B_LOC, L], BF16, tag="z2", name="z2") for _ in range(KC)]
                k2_all = [z3p.tile([P, B_LOC, L], BF16, tag="z3", name="z3") for _ in range(KC)]
                for mc in range(KC):
                    for b in range(B_LOC):
                        ps = psum.tile([P, L], F32, tag="ps", name="ps")
                        for kc in range(KC):
                            nc.tensor.matmul(
                                ps[:],
                                wq_sb[:, kc, mc * P : (mc + 1) * P],
                                l_all[kc][:, b, :],
                                start=(kc == 0),
                                stop=(kc == KC - 1),
                            )
                        nc.vector.scalar_tensor_tensor(
                            out=q2_all[mc][:, b, :], in0=ps[:], scalar=1.0,
                            in1=qbt_sb[:, mc, :], op0=ALU.mult, op1=ALU.add,
                            accum_out=az2[:, mc, b : b + 1],
                        )

                        ps = psum.tile([P, L], F32, tag="ps", name="ps")
                        for kc in range(KC):
                            nc.tensor.matmul(
                                ps[:],
                                wk_sb[:, kc, mc * P : (mc + 1) * P],
                                l_all[kc][:, b, :],
                                start=(kc == 0),
                                stop=(kc == KC - 1),
                            )
                        nc.vector.scalar_tensor_tensor(
                            out=k2_all[mc][:, b, :], in0=ps[:], scalar=1.0,
                            in1=kbt_sb[:, mc, :], op0=ALU.mult, op1=ALU.add,
                            accum_out=az3[:, mc, b : b + 1],
                        )
                    sumsq_chunk(q2_all[mc], azq2, mc)
                    sumsq_chunk(k2_all[mc], azq3, mc)

                pack23 = packp.tile([P, 2 * KC, 2], F32, tag="pk23", name="pack23")
                nc.vector.tensor_reduce(
                    out=pack23[:, 0:KC, 0], in_=az2[:], axis=AX.X, op=ALU.add
                )
                nc.vector.tensor_copy(pack23[:, 0:KC, 1], azq2[:])
                nc.vector.tensor_reduce(
                    out=pack23[:, KC : 2 * KC, 0], in_=az3[:], axis=AX.X, op=ALU.add
                )
                nc.vector.tensor_copy(pack23[:, KC : 2 * KC, 1], azq3[:])
                nc.vector.tensor_scalar_mul(pack23[:], pack23[:], 1.0 / NBL)
                g23 = allreduce(pack23, 4 * KC)

                # While the allreduce is in flight: transpose l on the PE
                # (stage 5 needs sequence-major l), drain via DVE copies.
                lstd_sb = {}
                for b in range(B_LOC):
                    for kc in range(KC):
                        pst = psumT.tile([P, E], BF16, tag="psT", name="psT")
                        for mc in range(KC):
                            nc.tensor.transpose(
                                pst[:, mc * P : (mc + 1) * P],
                                l_all[mc][:, b, kc * P : (kc + 1) * P],
                                ident[:],
                            )
                        lst = lsp.tile([P, E], BF16, tag="lstd", name="lstd")
                        nc.vector.tensor_copy(lst[:], pst[:])
                        lstd_sb[b, kc] = lst

                sc2, bi2 = affines(g23[:, 0 : 2 * KC], _G2, _B2)
                sc3, bi3 = affines(g23[:, 2 * KC : 4 * KC], _G3, _B3)

                for b in range(B_LOC):
                    for mc in range(KC):
                        elu_tile(
                            q2_all[mc][:, b, :],
                            sc2[:, mc : mc + 1], bi2[:, mc : mc + 1],
                        )
                        elu_tile(
                            k2_all[mc][:, b, :],
                            sc3[:, mc : mc + 1], bi3[:, mc : mc + 1],
                        )

                # ------------- Stage 4a: wT = (q2 @ k2^T)^T + wb^T -------------
                az4 = accp.tile([P, KC, B_LOC], F32, tag="az", name="az4")
                azq4 = accp.tile([P, KC], F32, tag="azq", name="azq4")
                wt_all = [wtp.tile([P, B_LOC, L], BF16, tag="wT", name="wT") for _ in range(KC)]
                wtmax = colp.tile([P, KC, B_LOC], F32, tag="mx", name="wtmax")
                for kc in range(KC):
                    for b in range(B_LOC):
                        ps = psum.tile([P, L], F32, tag="ps", name="ps")
                        for ec in range(KC):
                            nc.tensor.matmul(
                                ps[:],
                                k2_all[ec][:, b, kc * P : (kc + 1) * P],
                                q2_all[ec][:, b, :],
                                start=(ec == 0),
                                stop=(ec == KC - 1),
                            )
                        nc.vector.scalar_tensor_tensor(
                            out=wt_all[kc][:, b, :], in0=ps[:], scalar=1.0,
                            in1=wbt_sb[:, kc, :], op0=ALU.mult, op1=ALU.add,
                            accum_out=az4[:, kc, b : b + 1],
                        )
                    sumsq_chunk(wt_all[kc], azq4, kc)
                    # per-chunk max over (b, q) for the softmax stabilizer
                    nc.vector.tensor_reduce(
                        out=wtmax[:, kc, :], in_=wt_all[kc][:], axis=AX.X, op=ALU.max
                    )

            es_z.close()

            pack4 = make_pack(az4, azq4, "pack4")
            g4 = allreduce(pack4, KC * 2)
            # softmax(BN4(x)) over q: the BN4 bias cancels inside softmax, so
            # only the scale survives: softmax_q(sc4*x), stabilized with a
            # chunk-wide max folded into the exp bias.
            sc4, _ = affines(g4, _G4, _B4, want_bias=False)

            # ---------------- Stage 4b: softmax over q ----------------
            mxc = colp.tile([P, KC], F32, tag="mxc", name="mxc")
            nc.vector.tensor_reduce(out=mxc[:], in_=wtmax[:], axis=AX.X, op=ALU.max)
            mxs = colp.tile([P, KC], F32, tag="mxs", name="mxs")
            nc.vector.scalar_tensor_tensor(
                out=mxs[:], in0=mxc[:], scalar=-1.0, in1=sc4[:],
                op0=ALU.mult, op1=ALU.mult,
            )
            ssum = colp.tile([P, KC, B_LOC], F32, tag="ssum", name="ssum")
            rs = colp.tile([P, KC, B_LOC], F32, tag="rs", name="rs")
            for kc in range(KC):
                nc.scalar.activation(
                    out=wt_all[kc][:], in_=wt_all[kc][:], func=AF.Exp,
                    bias=mxs[:, kc : kc + 1], scale=sc4[:, kc : kc + 1],
                )
                nc.vector.tensor_reduce(
                    out=ssum[:, kc, :], in_=wt_all[kc][:], axis=AX.X, op=ALU.add
                )
            nc.vector.reciprocal(rs[:], ssum[:])
            for b in range(B_LOC):
                for kc in range(KC):
                    nc.vector.tensor_scalar_mul(
                        wt_all[kc][:, b, :], wt_all[kc][:, b, :], rs[:, kc, b : b + 1]
                    )

            # ---------------- Stage 5: out = w @ l, BN5 + ELU ----------------
            az5 = accp.tile([P, KC, B_LOC], F32, tag="az", name="az5")
            azq5 = accp.tile([P, KC], F32, tag="azq", name="azq5")
            es_s5 = ExitStack()
            if True:
                outp = es_s5.enter_context(tc.tile_pool(name="out", bufs=KC))
                out_all = [outp.tile([P, B_LOC, L], BF16, tag="out", name="out") for _ in range(KC)]
                for mc in range(KC):
                    for b in range(B_LOC):
                        ps = psum.tile([P, L], F32, tag="ps", name="ps")
                        for kc in range(KC):
                            nc.tensor.matmul(
                                ps[:],
                                lstd_sb[b, kc][:, mc * P : (mc + 1) * P],
                                wt_all[kc][:, b, :],
                                start=(kc == 0),
                                stop=(kc == KC - 1),
                            )
                        nc.vector.tensor_scalar(
                            out=out_all[mc][:, b, :], in0=ps[:],
                            scalar1=1.0, scalar2=0.0, op0=ALU.mult, op1=ALU.add,
                            accum_out=az5[:, mc, b : b + 1],
                        )
                    sumsq_chunk(out_all[mc], azq5, mc)

                pack5 = make_pack(az5, azq5, "pack5")
                g5 = allreduce(pack5, KC * 2)
                sc5, bi5 = affines(g5, _G5, _B5)
                for mc in range(KC):
                    for b in range(B_LOC):
                        elu_tile(
                            out_all[mc][:, b, :],
                            sc5[:, mc : mc + 1], bi5[:, mc : mc + 1],
                        )
                    nc.sync.dma_start(
                        out=out_d[:, mc * P : (mc + 1) * P, :].rearrange(
                            "b p l -> p b l"
                        ),
                        in_=out_all[mc][:],
                    )

                es_s5.close()
                es_l.close()

    _split_waits(nc)
    _CACHE["nc"] = nc
    return nc


def _pack_affine(vecs):
    cols = []
    for v in vecs:
        cols.append(np.ascontiguousarray(np.asarray(v, np.float32).reshape(KC, P).T))
    return np.ascontiguousarray(np.concatenate(cols, axis=1))


def kernel(m1, f, wq, wk, qb, kb, wb, g1, b1, g2, b2, g3, b3, g4, b4, g5, b5):
    BF = ml_dtypes.bfloat16
    m1 = np.asarray(m1, np.float32)
    nc = _build()
    m1t = np.ascontiguousarray(m1.transpose(0, 2, 1)).astype(BF)
    f_h = np.ascontiguousarray(np.asarray(f, np.float32)).astype(BF)
    wq_h = np.ascontiguousarray(np.asarray(wq, np.float32)).astype(BF)
    wk_h = np.ascontiguousarray(np.asarray(wk, np.float32)).astype(BF)
    qbt = np.ascontiguousarray(np.asarray(qb, np.float32).T)
    kbt = np.ascontiguousarray(np.asarray(kb, np.float32).T)
    wbt = np.ascontiguousarray(np.asarray(wb, np.float32).T)
    gpack = _pack_affine([g1, b1, g2, b2, g3, b3, g4, b4, g5, b5])

    shared = {
        "f": f_h, "wq": wq_h, "wk": wk_h,
        "qbt": qbt, "kbt": kbt, "wbt": wbt, "gpack": gpack,
    }
    in_maps = [
        {"m1t": np.ascontiguousarray(m1t[i * B_LOC : (i + 1) * B_LOC]), **shared}
        for i in range(N_CORES)
    ]
    trace = os.environ.get("KERNEL_TRACE") == "1"
    res = run_bass_kernel_spmd(nc, in_maps, list(range(N_CORES)), trace=trace)
    _CACHE["last_results"] = res

    out = np.empty((B, L, E), np.float32)
    for i in range(N_CORES):
        out[i * B_LOC : (i + 1) * B_LOC] = (
            res.results[i]["outt"].astype(np.float32).transpose(0, 2, 1)
        )
    return out


# revision 17
# speedup vs baseline: 2.4152x; 1.0024x over previous
"""Bass/Tile TRN2 kernel for nn_LAN_4320737100678 (dense transformer block).

Data-parallel over the batch axis across 8 NeuronCores (4 batches/core).
All activations are kept feature-major ([E, L] per batch) so that every
BatchNorm reduction and the softmax run along the free axis. The five
BatchNorm moment sets are globalized with four tiny in-kernel AllReduces
(BN2+BN3 share one round).

v3 (perf): all matmuls/transposes run in bf16 (1 cycle/row on the PE vs
4 for fp32 — the fp32 baseline was 100% TensorMatrix-bound). Engine
split keeps every engine under the PE's ~130us: the PSUM epilogue is one
fused DVE tensor_tensor_reduce (bias-add + sum accumulate), sum-of-
squares runs as batched per-chunk Square+accum on the Scalar engine,
ELU is exp (Scalar) + affine (DVE) + min/sub (Pool, SBUF-only) + max
(DVE), and the BN affine rsqrt is exp(-0.5*ln(var+eps)) so the Scalar
engine never leaves one activation-table set (the baseline burned 20us
on ACT_TABLE_LOAD thrash from Sqrt). The BN4 affine folds into the
softmax exp (bias cancels; scale rides the activation scale operand)
with one chunk-wide stabilizer max. Matmuls are emitted chunk-major so
the per-chunk stats passes overlap the matmul phase; l-transposes and
their PSUM drains fill the BN2/3 allreduce gap. Inputs load as a
handful of large batched DMAs.
"""

import os
import sys

sys.path.insert(0, "/opt/trn_rl_repo")

import numpy as np
import ml_dtypes

import concourse.bass as bass
import concourse.tile as tile
from concourse import mybir
from concourse.bass_utils import run_bass_kernel_spmd
from concourse.masks import make_identity

N_CORES = 8
B, L, E, W = 32, 512, 512, 5
S = W // 2
P = 128
KC = E // P            # feature chunks of 128
WK = W * KC            # stage-1 contraction tiles
B_LOC = B // N_CORES   # batches per core
EPS = 1e-3
F32 = mybir.dt.float32
BF16 = mybir.dt.bfloat16
AF = mybir.ActivationFunctionType
ALU = mybir.AluOpType
AX = mybir.AxisListType

# gpack column base offsets (each vector packed as [P, KC])
_G1, _B1, _G2, _B2, _G3, _B3, _G4, _B4, _G5, _B5 = (i * KC for i in range(10))

_MAX_CTRL_WAITS = 1


def _split_waits(nc, max_waits=_MAX_CTRL_WAITS):
    """walrus in this container encodes at most one sync-wait slot per
    instruction. Hoist extra waits onto same-engine NOPs inserted right
    before the owning instruction (same engine => executes first)."""
    for fn in nc.m.functions:
        for bb in fn.blocks:
            rebuilt = []
            changed = False
            for ins in bb.instructions:
                si = ins.sync_info
                if si is not None and len(si.on_wait) > max_waits:
                    waits = list(si.on_wait)
                    rest = waits[max_waits:]
                    for j in range(0, len(rest), max_waits):
                        nop = mybir.InstNoOp(
                            name=f"{ins.name}_wsplit{j}",
                            engine=ins.engine,
                            bass_nofuse=True,
                            sync_info=mybir.SyncInfo(
                                on_wait=rest[j : j + max_waits], on_update=[]
                            ),
                        )
                        rebuilt.append(nop)
                    ins.sync_info = mybir.SyncInfo(
                        on_wait=waits[:max_waits], on_update=list(si.on_update)
                    )
                    changed = True
                rebuilt.append(ins)
            if changed:
                bb.instructions = rebuilt


_CACHE = {}


def _build():
    if "nc" in _CACHE:
        return _CACHE["nc"]
    nc = bass.Bass("TRN2", target_bir_lowering=False, debug=False, num_devices=N_CORES)

    m1t_d = nc.dram_tensor("m1t", [B_LOC, E, L], BF16, kind="ExternalInput")
    f_d = nc.dram_tensor("f", [W * E, E], BF16, kind="ExternalInput")
    wq_d = nc.dram_tensor("wq", [E, E], BF16, kind="ExternalInput")
    wk_d = nc.dram_tensor("wk", [E, E], BF16, kind="ExternalInput")
    qbt_d = nc.dram_tensor("qbt", [E, L], F32, kind="ExternalInput")
    kbt_d = nc.dram_tensor("kbt", [E, L], F32, kind="ExternalInput")
    wbt_d = nc.dram_tensor("wbt", [L, L], F32, kind="ExternalInput")
    gp_d = nc.dram_tensor("gpack", [P, 10 * KC], F32, kind="ExternalInput")
    out_d = nc.dram_tensor("outt", [B_LOC, E, L], BF16, kind="ExternalOutput")

    groups = [list(range(N_CORES))]
    NBL = float(B * L)  # total elements per feature for BN moments

    from contextlib import ExitStack

    with tile.TileContext(nc) as tc:
        with (
            tc.tile_pool(name="const", bufs=1) as const,
            tc.tile_pool(name="bias", bufs=1) as biasp,
            tc.tile_pool(name="aff", bufs=12) as affp,
            tc.tile_pool(name="acc", bufs=24) as accp,
            tc.tile_pool(name="packs", bufs=8) as packp,
            tc.tile_pool(name="scr", bufs=16) as scr,
            tc.tile_pool(name="col", bufs=16) as colp,
            tc.tile_pool(name="elu", bufs=8) as elup,
            tc.tile_pool(name="junk", bufs=2) as junkp,
            tc.tile_pool(name="psum", bufs=4, space="PSUM") as psum,
            tc.tile_pool(name="psumT", bufs=4, space="PSUM") as psumT,
            tc.tile_pool(name="dram", bufs=8, space="DRAM") as dram,
        ):
            es_l = ExitStack()
            wtp = es_l.enter_context(tc.tile_pool(name="wT", bufs=KC))
            lp = es_l.enter_context(tc.tile_pool(name="l", bufs=KC))
            lsp = es_l.enter_context(tc.tile_pool(name="lstd", bufs=B_LOC * KC))
            gp = const.tile([P, 10 * KC], F32, tag="gp")
            nc.sync.dma_start(out=gp[:], in_=gp_d[:])
            ident = const.tile([P, P], BF16, tag="ident")
            make_identity(nc, ident[:])
            epst = const.tile([P, 1], F32, tag="eps")
            nc.vector.memset(epst[:], EPS)

            # Dummy collective at t=0: absorbs cross-core start skew while
            # the input DMAs stream, so the first real BN allreduce is fast.
            bar = packp.tile([P, 1], F32, tag="bar", name="bar")
            nc.vector.memset(bar[:], 0.0)

            # Batched parameter loads, spread across engine DMA queues.
            qbt_sb = biasp.tile([P, KC, L], F32, tag="qbt")
            nc.scalar.dma_start(
                out=qbt_sb[:], in_=qbt_d[:].rearrange("(kc p) l -> p kc l", p=P)
            )
            kbt_sb = biasp.tile([P, KC, L], F32, tag="kbt")
            nc.gpsimd.dma_start(
                out=kbt_sb[:], in_=kbt_d[:].rearrange("(kc p) l -> p kc l", p=P)
            )
            wbt_sb = biasp.tile([P, KC, L], F32, tag="wbt")
            nc.sync.dma_start(
                out=wbt_sb[:], in_=wbt_d[:].rearrange("(kc p) l -> p kc l", p=P)
            )

            def allreduce(pack, width):
                """pack: [P, width] f32 of per-core moment sums (pre-scaled).
                Returns the cross-core sum."""
                cc_in = dram.tile([P, width], F32, tag="cc", name="cc_in")
                cc_out = dram.tile([P, width], F32, tag="cc", name="cc_out")
                nc.gpsimd.dma_start(out=cc_in[:], in_=pack[:])
                nc.gpsimd.collective_compute(
                    "AllReduce",
                    ALU.add,
                    replica_groups=groups,
                    ins=[cc_in.opt()],
                    outs=[cc_out.opt()],
                )
                g = packp.tile([P, width], F32, tag="g", name="g")
                nc.gpsimd.dma_start(out=g[:], in_=cc_out[:])
                return g

            allreduce(bar, 1)  # skew-absorbing barrier, overlaps input DMAs

            def make_pack(az, azq, name):
                """az: [P, KC, B_LOC] f32 per-tile sums; azq: [P, KC] f32
                per-chunk sum of squares. Returns [P, KC, 2] pack of
                (mean, E[x^2]) scaled by 1/(B*L) ready for allreduce-add."""
                pack = packp.tile([P, KC, 2], F32, tag="pk", name=name)
                nc.vector.tensor_reduce(
                    out=pack[:, :, 0], in_=az[:], axis=AX.X, op=ALU.add
                )
                nc.vector.tensor_copy(pack[:, :, 1], azq[:])
                nc.vector.tensor_scalar_mul(pack[:], pack[:], 1.0 / NBL)
                return pack

            def affines(g, gcol, bcol, want_bias=True):
                """From allreduced [P, KC*2] (mean, E[x^2]) compute
                sc[P, KC] = gamma*rsqrt(var+eps), bi[P, KC] = beta - mean*sc.
                rsqrt as exp(-0.5*ln(var+eps)) keeps the Scalar engine inside
                one activation-table set."""
                gv = g.rearrange("p (c two) -> p c two", two=2)
                mean = gv[:, :, 0]
                ex2 = gv[:, :, 1]
                sq = scr.tile([P, KC], F32, tag="scr", name="sq2")
                nc.vector.tensor_mul(sq[:], mean, mean)
                var = scr.tile([P, KC], F32, tag="scr", name="var")
                nc.vector.tensor_sub(var[:], ex2, sq[:])
                lnv = scr.tile([P, KC], F32, tag="scr", name="lnv")
                nc.scalar.activation(out=lnv[:], in_=var[:], func=AF.Ln, bias=epst[:])
                nc.vector.tensor_scalar_mul(lnv[:], lnv[:], -0.5)
                rsq = scr.tile([P, KC], F32, tag="scr", name="rsq")
                nc.scalar.activation(out=rsq[:], in_=lnv[:], func=AF.Exp)
                sc = affp.tile([P, KC], F32, tag="aff", name="sc")
                nc.vector.tensor_mul(sc[:], rsq[:], gp[:, gcol : gcol + KC])
                if not want_bias:
                    return sc, None
                tb = scr.tile([P, KC], F32, tag="scr", name="tb")
                nc.vector.tensor_mul(tb[:], mean, sc[:])
                bi = affp.tile([P, KC], F32, tag="aff", name="bi")
                nc.vector.tensor_sub(bi[:], gp[:, bcol : bcol + KC], tb[:])
                return sc, bi

            def sumsq_chunk(z_chunk, azq, mc):
                """Scalar engine: one batched Square+accum pass over a whole
                [P, B_LOC, L] chunk -> azq[:, mc] = sum(z^2)."""
                junk = junkp.tile([P, B_LOC, L], BF16, tag="junk", name="junk")
                nc.scalar.activation(
                    out=junk[:], in_=z_chunk[:], func=AF.Square,
                    accum_out=azq[:, mc : mc + 1],
                )

            def elu_tile(zt, sc, bi):
                """zt <- elu(y), y = zt*sc + bi, via max(y, min(exp(y),1)-1).
                Scalar: exp.  DVE: y affine + final max.  Pool: min/sub."""
                e = elup.tile([P, L], BF16, tag="elu_e", name="elu_e")
                y = elup.tile([P, L], BF16, tag="elu_y", name="elu_y")
                nc.scalar.activation(out=e[:], in_=zt, func=AF.Exp, bias=bi, scale=sc)
                nc.vector.tensor_scalar(
                    out=y[:], in0=zt, scalar1=sc, scalar2=bi,
                    op0=ALU.mult, op1=ALU.add,
                )
                nc.vector.tensor_scalar(
                    out=e[:], in0=e[:], scalar1=1.0, scalar2=1.0,
                    op0=ALU.min, op1=ALU.subtract,
                )
                nc.vector.tensor_tensor(out=zt, in0=y[:], in1=e[:], op=ALU.max)

            # ---------------- Stage 1: z1 = unfold(m1) @ f + kb ----------------
            # l_all[mc]: [P, B_LOC, L] holds the full chunk across batches.
            l_all = [lp.tile([P, B_LOC, L], BF16, tag="l", name="l") for _ in range(KC)]
            az1 = accp.tile([P, KC, B_LOC], F32, tag="az", name="az1")
            azq1 = accp.tile([P, KC], F32, tag="azq", name="azq1")
            es_s1 = ExitStack()
            if True:
                fp = es_s1.enter_context(tc.tile_pool(name="f", bufs=1))
                mp = es_s1.enter_context(tc.tile_pool(name="m1", bufs=1))
                f_sb = fp.tile([P, WK, E], BF16, tag="f")
                m1_sb = mp.tile([P, B_LOC, KC, L + 2 * S], BF16, tag="m1")
                nc.vector.memset(m1_sb[:, :, :, 0:S], 0.0)
                nc.vector.memset(m1_sb[:, :, :, S + L : 2 * S + L], 0.0)
                # One DMA per filter tap / per batch, spread across the five
                # engine DGE queues so the transfers run in parallel.
                dma_engs = [nc.sync, nc.scalar, nc.gpsimd, nc.sync, nc.scalar]
                for w in range(W):
                    dma_engs[w].dma_start(
                        out=f_sb[:, w * KC : (w + 1) * KC, :],
                        in_=f_d[w * E : (w + 1) * E, :].rearrange(
                            "(kc p) e -> p kc e", p=P
                        ),
                    )
                for b in range(B_LOC):
                    dma_engs[b].dma_start(
                        out=m1_sb[:, b, :, S : S + L],
                        in_=m1t_d[b].rearrange("(kc p) l -> p kc l", p=P),
                    )

                # chunk-major so the per-chunk Square pass overlaps compute
                for mc in range(KC):
                    for b in range(B_LOC):
                        ps = psum.tile([P, L], F32, tag="ps", name="ps")
                        n = 0
                        for w in range(W):
                            for kc in range(KC):
                                nc.tensor.matmul(
                                    ps[:],
                                    f_sb[:, w * KC + kc, mc * P : (mc + 1) * P],
                                    m1_sb[:, b, kc, w : w + L],
                                    start=(n == 0),
                                    stop=(n == WK - 1),
                                )
                                n += 1
                        nc.vector.scalar_tensor_tensor(
                            out=l_all[mc][:, b, :], in0=ps[:], scalar=1.0,
                            in1=kbt_sb[:, mc, :], op0=ALU.mult, op1=ALU.add,
                            accum_out=az1[:, mc, b : b + 1],
                        )
                    sumsq_chunk(l_all[mc], azq1, mc)

            pack1 = make_pack(az1, azq1, "pack1")
            g1 = allreduce(pack1, KC * 2)
            sc1, bi1 = affines(g1, _G1, _B1)
            for b in range(B_LOC):
                for mc in range(KC):
                    elu_tile(
                        l_all[mc][:, b, :],
                        sc1[:, mc : mc + 1], bi1[:, mc : mc + 1],
                    )

            es_s1.close()

            # ------------- Stage 2/3: q2 = l@wq + qb, k2 = l@wk + kb -------------
            az2 = accp.tile([P, KC, B_LOC], F32, tag="az", name="az2")
            azq2 = accp.tile([P, KC], F32, tag="azq", name="azq2")
            az3 = accp.tile([P, KC, B_LOC], F32, tag="az", name="az3")
            azq3 = accp.tile([P, KC], F32, tag="azq", name="azq3")
            es_z = ExitStack()
            if True:
                z2p = es_z.enter_context(tc.tile_pool(name="z2", bufs=KC))
                z3p = es_z.enter_context(tc.tile_pool(name="z3", bufs=KC))
                wqkp = es_z.enter_context(tc.tile_pool(name="wqk", bufs=1))
                wq_sb = wqkp.tile([P, KC, E], BF16, tag="wq")
                nc.sync.dma_start(
                    out=wq_sb[:], in_=wq_d[:].rearrange("(kc p) e -> p kc e", p=P)
                )
                wk_sb = wqkp.tile([P, KC, E], BF16, tag="wk")
                nc.scalar.dma_start(
                    out=wk_sb[:], in_=wk_d[:].rearrange("(kc p) e -> p kc e", p=P)
                )

                q2_all = [z2p.tile([P, B_LOC, L], BF16, tag="z2", name="z2") for _ in range(KC)]
                k2_all = [z3p.tile([P, B_LOC, L], BF16, tag="z3", name="z3") for _ in range(KC)]
                for mc in range(KC):
                    for b in range(B_LOC):
                        ps = psum.tile([P, L], F32, tag="ps", name="ps")
                        for kc in range(KC):
                            nc.tensor.matmul(
                                ps[:],
                                wq_sb[:, kc, mc * P : (mc + 1) * P],
                                l_all[kc][:, b, :],
                                start=(kc == 0),
                                stop=(kc == KC - 1),
                            )
                        nc.vector.scalar_tensor_tensor(
                            out=q2_all[mc][:, b, :], in0=ps[:], scalar=1.0,
                            in1=qbt_sb[:, mc, :], op0=ALU.mult, op1=ALU.add,
                            accum_out=az2[:, mc, b : b + 1],
                        )

                        ps = psum.tile([P, L], F32, tag="ps", name="ps")
                        for kc in range(KC):
                            nc.tensor.matmul(
                                ps[:],
                                wk_sb[:, kc, mc * P : (mc + 1) * P],
                                l_all[kc][:, b, :],
                                start=(kc == 0),
                                stop=(kc == KC - 1),
                            )
                        nc.vector.scalar_tensor_tensor(
                            out=k2_all[mc][:, b, :], in0=ps[:], scalar=1.0,
                            in1=kbt_sb[:, mc, :], op0=ALU.mult, op1=ALU.add,
                            accum_out=az3[:, mc, b : b + 1],
                        )
                    sumsq_chunk(q2_all[mc], azq2, mc)
                    sumsq_chunk(k2_all[mc], azq3, mc)

                pack23 = packp.tile([P, 2 * KC, 2], F32, tag="pk23", name="pack23")
                nc.vector.tensor_reduce(
                    out=pack23[:, 0:KC, 0], in_=az2[:], axis=AX.X, op=ALU.add
                )
                nc.vector.tensor_copy(pack23[:, 0:KC, 1], azq2[:])
                nc.vector.tensor_reduce(
                    out=pack23[:, KC : 2 * KC, 0], in_=az3[:], axis=AX.X, op=ALU.add
                )
                nc.vector.tensor_copy(pack23[:, KC : 2 * KC, 1], azq3[:])
                nc.vector.tensor_scalar_mul(pack23[:], pack23[:], 1.0 / NBL)
                g23 = allreduce(pack23, 4 * KC)

                # While the allreduce is in flight: transpose l on the PE
                # (stage 5 needs sequence-major l), drain via DVE copies.
                lstd_sb = {}
                for b in range(B_LOC):
                    for kc in range(KC):
                        pst = psumT.tile([P, E], BF16, tag="psT", name="psT")
                        for mc in range(KC):
                            nc.tensor.transpose(
                                pst[:, mc * P : (mc + 1) * P],
                                l_all[mc][:, b, kc * P : (kc + 1) * P],
                                ident[:],
                            )
                        lst = lsp.tile([P, E], BF16, tag="lstd", name="lstd")
                        nc.vector.tensor_copy(lst[:], pst[:])
                        lstd_sb[b, kc] = lst

                sc2, bi2 = affines(g23[:, 0 : 2 * KC], _G2, _B2)
                sc3, bi3 = affines(g23[:, 2 * KC : 4 * KC], _G3, _B3)

                for b in range(B_LOC):
                    for mc in range(KC):
                        elu_tile(
                            q2_all[mc][:, b, :],
                            sc2[:, mc : mc + 1], bi2[:, mc : mc + 1],
                        )
                        elu_tile(
                            k2_all[mc][:, b, :],
                            sc3[:, mc : mc + 1], bi3[:, mc : mc + 1],
                        )

                # ------------- Stage 4a: wT = (q2 @ k2^T)^T + wb^T -------------
                az4 = accp.tile([P, KC, B_LOC], F32, tag="az", name="az4")
                azq4 = accp.tile([P, KC], F32, tag="azq", name="azq4")
                wt_all = [wtp.tile([P, B_LOC, L], BF16, tag="wT", name="wT") for _ in range(KC)]
                wtmax = colp.tile([P, KC, B_LOC], F32, tag="mx", name="wtmax")
                for kc in range(KC):
                    for b in range(B_LOC):
                        ps = psum.tile([P, L], F32, tag="ps", name="ps")
                        for ec in range(KC):
                            nc.tensor.matmul(
                                ps[:],
                                k2_all[ec][:, b, kc * P : (kc + 1) * P],
                                q2_all[ec][:, b, :],
                                start=(ec == 0),
                                stop=(ec == KC - 1),
                            )
                        nc.vector.scalar_tensor_tensor(
                            out=wt_all[kc][:, b, :], in0=ps[:], scalar=1.0,
                            in1=wbt_sb[:, kc, :], op0=ALU.mult, op1=ALU.add,
                            accum_out=az4[:, kc, b : b + 1],
                        )
                    sumsq_chunk(wt_all[kc], azq4, kc)
                    # per-chunk max over (b, q) for the softmax stabilizer
                    nc.vector.tensor_reduce(
                        out=wtmax[:, kc, :], in_=wt_all[kc][:], axis=AX.X, op=ALU.max
                    )

            es_z.close()

            pack4 = make_pack(az4, azq4, "pack4")
            g4 = allreduce(pack4, KC * 2)
            # softmax(BN4(x)) over q: the BN4 bias cancels inside softmax, so
            # only the scale survives: softmax_q(sc4*x), stabilized with a
            # chunk-wide max folded into the exp bias.
            sc4, _ = affines(g4, _G4, _B4, want_bias=False)

            # ---------------- Stage 4b: softmax over q ----------------
            mxc = colp.tile([P, KC], F32, tag="mxc", name="mxc")
            nc.vector.tensor_reduce(out=mxc[:], in_=wtmax[:], axis=AX.X, op=ALU.max)
            mxs = colp.tile([P, KC], F32, tag="mxs", name="mxs")
            nc.vector.scalar_tensor_tensor(
                out=mxs[:], in0=mxc[:], scalar=-1.0, in1=sc4[:],
                op0=ALU.mult, op1=ALU.mult,
            )
            # Per-tile exp (+row-sum accumulator) so batch 0 finishes its four
            # chunks quickly and stage 5 can start while later batches exp.
            ssum = colp.tile([P, KC, B_LOC], F32, tag="ssum", name="ssum")
            rs = colp.tile([P, KC, B_LOC], F32, tag="rs", name="rs")
            for b in range(B_LOC):
                for kc in range(KC):
                    nc.scalar.activation(
                        out=wt_all[kc][:, b, :], in_=wt_all[kc][:, b, :], func=AF.Exp,
                        bias=mxs[:, kc : kc + 1], scale=sc4[:, kc : kc + 1],
                        accum_out=ssum[:, kc, b : b + 1],
                    )
                    nc.vector.reciprocal(rs[:, kc, b : b + 1], ssum[:, kc, b : b + 1])
                    nc.vector.tensor_scalar_mul(
                        wt_all[kc][:, b, :], wt_all[kc][:, b, :], rs[:, kc, b : b + 1]
                    )

            # ---------------- Stage 5: out = w @ l, BN5 + ELU ----------------
            az5 = accp.tile([P, KC, B_LOC], F32, tag="az", name="az5")
            azq5 = accp.tile([P, KC], F32, tag="azq", name="azq5")
            es_s5 = ExitStack()
            if True:
                outp = es_s5.enter_context(tc.tile_pool(name="out", bufs=KC))
                out_all = [outp.tile([P, B_LOC, L], BF16, tag="out", name="out") for _ in range(KC)]
                for mc in range(KC):
                    for b in range(B_LOC):
                        ps = psum.tile([P, L], F32, tag="ps", name="ps")
                        for kc in range(KC):
                            nc.tensor.matmul(
                                ps[:],
                                lstd_sb[b, kc][:, mc * P : (mc + 1) * P],
                                wt_all[kc][:, b, :],
                                start=(kc == 0),
                                stop=(kc == KC - 1),
                            )
                        nc.vector.tensor_scalar(
                            out=out_all[mc][:, b, :], in0=ps[:],
                            scalar1=1.0, scalar2=0.0, op0=ALU.mult, op1=ALU.add,
                            accum_out=az5[:, mc, b : b + 1],
                        )
                    sumsq_chunk(out_all[mc], azq5, mc)

                pack5 = make_pack(az5, azq5, "pack5")
                g5 = allreduce(pack5, KC * 2)
                sc5, bi5 = affines(g5, _G5, _B5)
                for mc in range(KC):
                    for b in range(B_LOC):
                        elu_tile(
                            out_all[mc][:, b, :],
                            sc5[:, mc : mc + 1], bi5[:, mc : mc + 1],
                        )
                    nc.sync.dma_start(
                        out=out_d[:, mc * P : (mc + 1) * P, :].rearrange(
                            "b p l -> p b l"
                        ),
                        in_=out_all[mc][:],
                    )

                es_s5.close()
                es_l.close()

    _split_waits(nc)
    _CACHE["nc"] = nc
    return nc


def _pack_affine(vecs):
    cols = []
    for v in vecs:
        cols.append(np.ascontiguousarray(np.asarray(v, np.float32).reshape(KC, P).T))
    return np.ascontiguousarray(np.concatenate(cols, axis=1))


def kernel(m1, f, wq, wk, qb, kb, wb, g1, b1, g2, b2, g3, b3, g4, b4, g5, b5):
    BF = ml_dtypes.bfloat16
    m1 = np.asarray(m1, np.float32)
    nc = _build()
    m1t = np.ascontiguousarray(m1.transpose(0, 2, 1)).astype(BF)
    f_h = np.ascontiguousarray(np.asarray(f, np.float32)).astype(BF)
    wq_h = np.ascontiguousarray(np.asarray(wq, np.float32)).astype(BF)
    wk_h = np.ascontiguousarray(np.asarray(wk, np.float32)).astype(BF)
    qbt = np.ascontiguousarray(np.asarray(qb, np.float32).T)
    kbt = np.ascontiguousarray(np.asarray(kb, np.float32).T)
    wbt = np.ascontiguousarray(np.asarray(wb, np.float32).T)
    gpack = _pack_affine([g1, b1, g2, b2, g3, b3, g4, b4, g5, b5])

    shared = {
        "f": f_h, "wq": wq_h, "wk": wk_h,
        "qbt": qbt, "kbt": kbt, "wbt": wbt, "gpack": gpack,
    }
    in_maps = [
        {"m1t": np.ascontiguousarray(m1t[i * B_LOC : (i + 1) * B_LOC]), **shared}
        for i in range(N_CORES)
    ]
    trace = os.environ.get("KERNEL_TRACE") == "1"
    res = run_bass_kernel_spmd(nc, in_maps, list(range(N_CORES)), trace=trace)
    _CACHE["last_results"] = res

    out = np.empty((B, L, E), np.float32)
    for i in range(N_CORES):
        out[i * B_LOC : (i + 1) * B_LOC] = (
            res.results[i]["outt"].astype(np.float32).transpose(0, 2, 1)
        )
    return out


# revision 18
# speedup vs baseline: 2.5264x; 1.0461x over previous
"""Bass/Tile TRN2 kernel for nn_LAN_4320737100678 (dense transformer block).

Data-parallel over the batch axis across 8 NeuronCores (4 batches/core).
All activations are kept feature-major ([E, L] per batch) so that every
BatchNorm reduction and the softmax run along the free axis. The five
BatchNorm moment sets are globalized with four tiny in-kernel AllReduces
(BN2+BN3 share one round).

v3 (perf): all matmuls/transposes run in bf16 (1 cycle/row on the PE vs
4 for fp32 — the fp32 baseline was 100% TensorMatrix-bound). Engine
split keeps every engine under the PE's ~130us: the PSUM epilogue is one
fused DVE tensor_tensor_reduce (bias-add + sum accumulate), sum-of-
squares runs as batched per-chunk Square+accum on the Scalar engine,
ELU is exp (Scalar) + affine (DVE) + min/sub (Pool, SBUF-only) + max
(DVE), and the BN affine rsqrt is exp(-0.5*ln(var+eps)) so the Scalar
engine never leaves one activation-table set (the baseline burned 20us
on ACT_TABLE_LOAD thrash from Sqrt). The BN4 affine folds into the
softmax exp (bias cancels; scale rides the activation scale operand)
with one chunk-wide stabilizer max. Matmuls are emitted chunk-major so
the per-chunk stats passes overlap the matmul phase; l-transposes and
their PSUM drains fill the BN2/3 allreduce gap. Inputs load as a
handful of large batched DMAs.
"""

import os
import sys

sys.path.insert(0, "/opt/trn_rl_repo")

import numpy as np
import ml_dtypes

import concourse.bass as bass
import concourse.tile as tile
from concourse import mybir
from concourse.bass_utils import run_bass_kernel_spmd
from concourse.masks import make_identity

N_CORES = 8
B, L, E, W = 32, 512, 512, 5
S = W // 2
P = 128
KC = E // P            # feature chunks of 128
WK = W * KC            # stage-1 contraction tiles
B_LOC = B // N_CORES   # batches per core
EPS = 1e-3
F32 = mybir.dt.float32
BF16 = mybir.dt.bfloat16
AF = mybir.ActivationFunctionType
ALU = mybir.AluOpType
AX = mybir.AxisListType

# gpack column base offsets (each vector packed as [P, KC])
_G1, _B1, _G2, _B2, _G3, _B3, _G4, _B4, _G5, _B5 = (i * KC for i in range(10))

_MAX_CTRL_WAITS = 1


def _split_waits(nc, max_waits=_MAX_CTRL_WAITS):
    """walrus in this container encodes at most one sync-wait slot per
    instruction. Hoist extra waits onto same-engine NOPs inserted right
    before the owning instruction (same engine => executes first)."""
    for fn in nc.m.functions:
        for bb in fn.blocks:
            rebuilt = []
            changed = False
            for ins in bb.instructions:
                si = ins.sync_info
                if si is not None and len(si.on_wait) > max_waits:
                    waits = list(si.on_wait)
                    rest = waits[max_waits:]
                    for j in range(0, len(rest), max_waits):
                        nop = mybir.InstNoOp(
                            name=f"{ins.name}_wsplit{j}",
                            engine=ins.engine,
                            bass_nofuse=True,
                            sync_info=mybir.SyncInfo(
                                on_wait=rest[j : j + max_waits], on_update=[]
                            ),
                        )
                        rebuilt.append(nop)
                    ins.sync_info = mybir.SyncInfo(
                        on_wait=waits[:max_waits], on_update=list(si.on_update)
                    )
                    changed = True
                rebuilt.append(ins)
            if changed:
                bb.instructions = rebuilt


_CACHE = {}


def _build():
    if "nc" in _CACHE:
        return _CACHE["nc"]
    nc = bass.Bass("TRN2", target_bir_lowering=False, debug=False, num_devices=N_CORES)

    m1t_d = nc.dram_tensor("m1t", [B_LOC, E, L], BF16, kind="ExternalInput")
    f_d = nc.dram_tensor("f", [W * E, E], BF16, kind="ExternalInput")
    wq_d = nc.dram_tensor("wq", [E, E], BF16, kind="ExternalInput")
    wk_d = nc.dram_tensor("wk", [E, E], BF16, kind="ExternalInput")
    qbt_d = nc.dram_tensor("qbt", [E, L], F32, kind="ExternalInput")
    kbt_d = nc.dram_tensor("kbt", [E, L], F32, kind="ExternalInput")
    wbt_d = nc.dram_tensor("wbt", [L, L], F32, kind="ExternalInput")
    gp_d = nc.dram_tensor("gpack", [P, 10 * KC], F32, kind="ExternalInput")
    out_d = nc.dram_tensor("outt", [B_LOC, E, L], BF16, kind="ExternalOutput")

    groups = [list(range(N_CORES))]
    NBL = float(B * L)  # total elements per feature for BN moments

    from contextlib import ExitStack

    with tile.TileContext(nc) as tc:
        with (
            tc.tile_pool(name="const", bufs=1) as const,
            tc.tile_pool(name="bias", bufs=1) as biasp,
            tc.tile_pool(name="aff", bufs=12) as affp,
            tc.tile_pool(name="acc", bufs=24) as accp,
            tc.tile_pool(name="packs", bufs=8) as packp,
            tc.tile_pool(name="scr", bufs=16) as scr,
            tc.tile_pool(name="col", bufs=16) as colp,
            tc.tile_pool(name="elu", bufs=8) as elup,
            tc.tile_pool(name="junk", bufs=2) as junkp,
            tc.tile_pool(name="psum", bufs=4, space="PSUM") as psum,
            tc.tile_pool(name="psumT", bufs=4, space="PSUM") as psumT,
            tc.tile_pool(name="dram", bufs=8, space="DRAM") as dram,
        ):
            es_l = ExitStack()
            wtp = es_l.enter_context(tc.tile_pool(name="wT", bufs=KC))
            lp = es_l.enter_context(tc.tile_pool(name="l", bufs=KC))
            lsp = es_l.enter_context(tc.tile_pool(name="lstd", bufs=B_LOC * KC))
            gp = const.tile([P, 10 * KC], F32, tag="gp")
            ident = const.tile([P, P], BF16, tag="ident")
            make_identity(nc, ident[:])
            epst = const.tile([P, 1], F32, tag="eps")
            nc.vector.memset(epst[:], EPS)

            # Dummy collective at t=0: absorbs cross-core start skew while
            # the input DMAs stream, so the first real BN allreduce is fast.
            bar = packp.tile([P, 1], F32, tag="bar", name="bar")
            nc.vector.memset(bar[:], 0.0)

            # Bias tiles (DMAs issued below, after the critical f/m1 loads).
            qbt_sb = biasp.tile([P, KC, L], F32, tag="qbt")
            kbt_sb = biasp.tile([P, KC, L], F32, tag="kbt")
            wbt_sb = biasp.tile([P, KC, L], F32, tag="wbt")

            def allreduce(pack, width):
                """pack: [P, width] f32 of per-core moment sums (pre-scaled).
                Returns the cross-core sum."""
                cc_in = dram.tile([P, width], F32, tag="cc", name="cc_in")
                cc_out = dram.tile([P, width], F32, tag="cc", name="cc_out")
                nc.gpsimd.dma_start(out=cc_in[:], in_=pack[:])
                nc.gpsimd.collective_compute(
                    "AllReduce",
                    ALU.add,
                    replica_groups=groups,
                    ins=[cc_in.opt()],
                    outs=[cc_out.opt()],
                )
                g = packp.tile([P, width], F32, tag="g", name="g")
                nc.gpsimd.dma_start(out=g[:], in_=cc_out[:])
                return g

            allreduce(bar, 1)  # skew-absorbing barrier, overlaps input DMAs

            def make_pack(az, azq, name):
                """az: [P, KC, B_LOC] f32 per-tile sums; azq: [P, KC] f32
                per-chunk sum of squares. Returns [P, KC, 2] pack of
                (mean, E[x^2]) scaled by 1/(B*L) ready for allreduce-add."""
                pack = packp.tile([P, KC, 2], F32, tag="pk", name=name)
                nc.vector.tensor_reduce(
                    out=pack[:, :, 0], in_=az[:], axis=AX.X, op=ALU.add
                )
                nc.vector.tensor_copy(pack[:, :, 1], azq[:])
                nc.vector.tensor_scalar_mul(pack[:], pack[:], 1.0 / NBL)
                return pack

            def affines(g, gcol, bcol, want_bias=True):
                """From allreduced [P, KC*2] (mean, E[x^2]) compute
                sc[P, KC] = gamma*rsqrt(var+eps), bi[P, KC] = beta - mean*sc.
                rsqrt as exp(-0.5*ln(var+eps)) keeps the Scalar engine inside
                one activation-table set."""
                gv = g.rearrange("p (c two) -> p c two", two=2)
                mean = gv[:, :, 0]
                ex2 = gv[:, :, 1]
                sq = scr.tile([P, KC], F32, tag="scr", name="sq2")
                nc.vector.tensor_mul(sq[:], mean, mean)
                var = scr.tile([P, KC], F32, tag="scr", name="var")
                nc.vector.tensor_sub(var[:], ex2, sq[:])
                lnv = scr.tile([P, KC], F32, tag="scr", name="lnv")
                nc.scalar.activation(out=lnv[:], in_=var[:], func=AF.Ln, bias=epst[:])
                nc.vector.tensor_scalar_mul(lnv[:], lnv[:], -0.5)
                rsq = scr.tile([P, KC], F32, tag="scr", name="rsq")
                nc.scalar.activation(out=rsq[:], in_=lnv[:], func=AF.Exp)
                sc = affp.tile([P, KC], F32, tag="aff", name="sc")
                nc.vector.tensor_mul(sc[:], rsq[:], gp[:, gcol : gcol + KC])
                if not want_bias:
                    return sc, None
                tb = scr.tile([P, KC], F32, tag="scr", name="tb")
                nc.vector.tensor_mul(tb[:], mean, sc[:])
                bi = affp.tile([P, KC], F32, tag="aff", name="bi")
                nc.vector.tensor_sub(bi[:], gp[:, bcol : bcol + KC], tb[:])
                return sc, bi

            def sumsq_chunk(z_chunk, azq, mc):
                """Scalar engine: one batched Square+accum pass over a whole
                [P, B_LOC, L] chunk -> azq[:, mc] = sum(z^2)."""
                junk = junkp.tile([P, B_LOC, L], BF16, tag="junk", name="junk")
                nc.scalar.activation(
                    out=junk[:], in_=z_chunk[:], func=AF.Square,
                    accum_out=azq[:, mc : mc + 1],
                )

            def elu_tile(zt, sc, bi):
                """zt <- elu(y), y = zt*sc + bi, via max(y, min(exp(y),1)-1).
                Scalar: exp.  DVE: y affine + final max.  Pool: min/sub."""
                e = elup.tile([P, L], BF16, tag="elu_e", name="elu_e")
                y = elup.tile([P, L], BF16, tag="elu_y", name="elu_y")
                nc.scalar.activation(out=e[:], in_=zt, func=AF.Exp, bias=bi, scale=sc)
                nc.vector.tensor_scalar(
                    out=y[:], in0=zt, scalar1=sc, scalar2=bi,
                    op0=ALU.mult, op1=ALU.add,
                )
                nc.vector.tensor_scalar(
                    out=e[:], in0=e[:], scalar1=1.0, scalar2=1.0,
                    op0=ALU.min, op1=ALU.subtract,
                )
                nc.vector.tensor_tensor(out=zt, in0=y[:], in1=e[:], op=ALU.max)

            # ---------------- Stage 1: z1 = unfold(m1) @ f + kb ----------------
            # l_all[mc]: [P, B_LOC, L] holds the full chunk across batches.
            l_all = [lp.tile([P, B_LOC, L], BF16, tag="l", name="l") for _ in range(KC)]
            az1 = accp.tile([P, KC, B_LOC], F32, tag="az", name="az1")
            azq1 = accp.tile([P, KC], F32, tag="azq", name="azq1")
            es_s1 = ExitStack()
            if True:
                fp = es_s1.enter_context(tc.tile_pool(name="f", bufs=1))
                mp = es_s1.enter_context(tc.tile_pool(name="m1", bufs=1))
                f_sb = fp.tile([P, WK, E], BF16, tag="f")
                m1_sb = mp.tile([P, B_LOC, KC, L + 2 * S], BF16, tag="m1")
                nc.vector.memset(m1_sb[:, :, :, 0:S], 0.0)
                nc.vector.memset(m1_sb[:, :, :, S + L : 2 * S + L], 0.0)

                # Queue plan (3 DGE queues, ~24ns/1KB-row each): the first
                # stage-1 tile needs every f tap + m1[b0]; kb arrives just in
                # time for its epilogue; everything else trails.
                def f_dma(eng, w):
                    eng.dma_start(
                        out=f_sb[:, w * KC : (w + 1) * KC, :],
                        in_=f_d[w * E : (w + 1) * E, :].rearrange(
                            "(kc p) e -> p kc e", p=P
                        ),
                    )

                def m1_dma(eng, b):
                    eng.dma_start(
                        out=m1_sb[:, b, :, S : S + L],
                        in_=m1t_d[b].rearrange("(kc p) l -> p kc l", p=P),
                    )

                f_dma(nc.sync, 0)
                f_dma(nc.scalar, 1)
                m1_dma(nc.gpsimd, 0)
                f_dma(nc.sync, 3)
                f_dma(nc.scalar, 4)
                f_dma(nc.gpsimd, 2)
                nc.sync.dma_start(
                    out=kbt_sb[:], in_=kbt_d[:].rearrange("(kc p) l -> p kc l", p=P)
                )
                m1_dma(nc.scalar, 1)
                m1_dma(nc.gpsimd, 3)
                m1_dma(nc.scalar, 2)
                nc.sync.dma_start(out=gp[:], in_=gp_d[:])
                nc.gpsimd.dma_start(
                    out=qbt_sb[:], in_=qbt_d[:].rearrange("(kc p) l -> p kc l", p=P)
                )
                nc.scalar.dma_start(
                    out=wbt_sb[:], in_=wbt_d[:].rearrange("(kc p) l -> p kc l", p=P)
                )

                # chunk-major so the per-chunk Square pass overlaps compute
                for mc in range(KC):
                    for b in range(B_LOC):
                        ps = psum.tile([P, L], F32, tag="ps", name="ps")
                        n = 0
                        for w in range(W):
                            for kc in range(KC):
                                nc.tensor.matmul(
                                    ps[:],
                                    f_sb[:, w * KC + kc, mc * P : (mc + 1) * P],
                                    m1_sb[:, b, kc, w : w + L],
                                    start=(n == 0),
                                    stop=(n == WK - 1),
                                )
                                n += 1
                        nc.vector.scalar_tensor_tensor(
                            out=l_all[mc][:, b, :], in0=ps[:], scalar=1.0,
                            in1=kbt_sb[:, mc, :], op0=ALU.mult, op1=ALU.add,
                            accum_out=az1[:, mc, b : b + 1],
                        )
                    sumsq_chunk(l_all[mc], azq1, mc)

            pack1 = make_pack(az1, azq1, "pack1")
            g1 = allreduce(pack1, KC * 2)
            sc1, bi1 = affines(g1, _G1, _B1)
            for b in range(B_LOC):
                for mc in range(KC):
                    elu_tile(
                        l_all[mc][:, b, :],
                        sc1[:, mc : mc + 1], bi1[:, mc : mc + 1],
                    )

            es_s1.close()

            # ------------- Stage 2/3: q2 = l@wq + qb, k2 = l@wk + kb -------------
            az2 = accp.tile([P, KC, B_LOC], F32, tag="az", name="az2")
            azq2 = accp.tile([P, KC], F32, tag="azq", name="azq2")
            az3 = accp.tile([P, KC, B_LOC], F32, tag="az", name="az3")
            azq3 = accp.tile([P, KC], F32, tag="azq", name="azq3")
            es_z = ExitStack()
            if True:
                z2p = es_z.enter_context(tc.tile_pool(name="z2", bufs=KC))
                z3p = es_z.enter_context(tc.tile_pool(name="z3", bufs=KC))
                wqkp = es_z.enter_context(tc.tile_pool(name="wqk", bufs=1))
                wq_sb = wqkp.tile([P, KC, E], BF16, tag="wq")
                nc.sync.dma_start(
                    out=wq_sb[:], in_=wq_d[:].rearrange("(kc p) e -> p kc e", p=P)
                )
                wk_sb = wqkp.tile([P, KC, E], BF16, tag="wk")
                nc.scalar.dma_start(
                    out=wk_sb[:], in_=wk_d[:].rearrange("(kc p) e -> p kc e", p=P)
                )

                q2_all = [z2p.tile([P, B_LOC, L], BF16, tag="z2", name="z2") for _ in range(KC)]
                k2_all = [z3p.tile([P, B_LOC, L], BF16, tag="z3", name="z3") for _ in range(KC)]
                for mc in range(KC):
                    for b in range(B_LOC):
                        ps = psum.tile([P, L], F32, tag="ps", name="ps")
                        for kc in range(KC):
                            nc.tensor.matmul(
                                ps[:],
                                wq_sb[:, kc, mc * P : (mc + 1) * P],
                                l_all[kc][:, b, :],
                                start=(kc == 0),
                                stop=(kc == KC - 1),
                            )
                        nc.vector.scalar_tensor_tensor(
                            out=q2_all[mc][:, b, :], in0=ps[:], scalar=1.0,
                            in1=qbt_sb[:, mc, :], op0=ALU.mult, op1=ALU.add,
                            accum_out=az2[:, mc, b : b + 1],
                        )

                        ps = psum.tile([P, L], F32, tag="ps", name="ps")
                        for kc in range(KC):
                            nc.tensor.matmul(
                                ps[:],
                                wk_sb[:, kc, mc * P : (mc + 1) * P],
                                l_all[kc][:, b, :],
                                start=(kc == 0),
                                stop=(kc == KC - 1),
                            )
                        nc.vector.scalar_tensor_tensor(
                            out=k2_all[mc][:, b, :], in0=ps[:], scalar=1.0,
                            in1=kbt_sb[:, mc, :], op0=ALU.mult, op1=ALU.add,
                            accum_out=az3[:, mc, b : b + 1],
                        )
                    sumsq_chunk(q2_all[mc], azq2, mc)
                    sumsq_chunk(k2_all[mc], azq3, mc)

                pack23 = packp.tile([P, 2 * KC, 2], F32, tag="pk23", name="pack23")
                nc.vector.tensor_reduce(
                    out=pack23[:, 0:KC, 0], in_=az2[:], axis=AX.X, op=ALU.add
                )
                nc.vector.tensor_copy(pack23[:, 0:KC, 1], azq2[:])
                nc.vector.tensor_reduce(
                    out=pack23[:, KC : 2 * KC, 0], in_=az3[:], axis=AX.X, op=ALU.add
                )
                nc.vector.tensor_copy(pack23[:, KC : 2 * KC, 1], azq3[:])
                nc.vector.tensor_scalar_mul(pack23[:], pack23[:], 1.0 / NBL)
                g23 = allreduce(pack23, 4 * KC)

                # While the allreduce is in flight: transpose l on the PE
                # (stage 5 needs sequence-major l), drain via DVE copies.
                lstd_sb = {}
                for b in range(B_LOC):
                    for kc in range(KC):
                        pst = psumT.tile([P, E], BF16, tag="psT", name="psT")
                        for mc in range(KC):
                            nc.tensor.transpose(
                                pst[:, mc * P : (mc + 1) * P],
                                l_all[mc][:, b, kc * P : (kc + 1) * P],
                                ident[:],
                            )
                        lst = lsp.tile([P, E], BF16, tag="lstd", name="lstd")
                        nc.vector.tensor_copy(lst[:], pst[:])
                        lstd_sb[b, kc] = lst

                sc2, bi2 = affines(g23[:, 0 : 2 * KC], _G2, _B2)
                sc3, bi3 = affines(g23[:, 2 * KC : 4 * KC], _G3, _B3)

                for b in range(B_LOC):
                    for mc in range(KC):
                        elu_tile(
                            q2_all[mc][:, b, :],
                            sc2[:, mc : mc + 1], bi2[:, mc : mc + 1],
                        )
                        elu_tile(
                            k2_all[mc][:, b, :],
                            sc3[:, mc : mc + 1], bi3[:, mc : mc + 1],
                        )

                # ------------- Stage 4a: wT = (q2 @ k2^T)^T + wb^T -------------
                az4 = accp.tile([P, KC, B_LOC], F32, tag="az", name="az4")
                azq4 = accp.tile([P, KC], F32, tag="azq", name="azq4")
                wt_all = [wtp.tile([P, B_LOC, L], BF16, tag="wT", name="wT") for _ in range(KC)]
                wtmax = colp.tile([P, KC, B_LOC], F32, tag="mx", name="wtmax")
                for kc in range(KC):
                    for b in range(B_LOC):
                        ps = psum.tile([P, L], F32, tag="ps", name="ps")
                        for ec in range(KC):
                            nc.tensor.matmul(
                                ps[:],
                                k2_all[ec][:, b, kc * P : (kc + 1) * P],
                                q2_all[ec][:, b, :],
                                start=(ec == 0),
                                stop=(ec == KC - 1),
                            )
                        nc.vector.scalar_tensor_tensor(
                            out=wt_all[kc][:, b, :], in0=ps[:], scalar=1.0,
                            in1=wbt_sb[:, kc, :], op0=ALU.mult, op1=ALU.add,
                            accum_out=az4[:, kc, b : b + 1],
                        )
                    sumsq_chunk(wt_all[kc], azq4, kc)
                    # per-chunk max over (b, q) for the softmax stabilizer
                    nc.vector.tensor_reduce(
                        out=wtmax[:, kc, :], in_=wt_all[kc][:], axis=AX.X, op=ALU.max
                    )

            es_z.close()

            pack4 = make_pack(az4, azq4, "pack4")
            g4 = allreduce(pack4, KC * 2)
            # softmax(BN4(x)) over q: the BN4 bias cancels inside softmax, so
            # only the scale survives: softmax_q(sc4*x), stabilized with a
            # chunk-wide max folded into the exp bias.
            sc4, _ = affines(g4, _G4, _B4, want_bias=False)

            # ---------------- Stage 4b: softmax over q ----------------
            mxc = colp.tile([P, KC], F32, tag="mxc", name="mxc")
            nc.vector.tensor_reduce(out=mxc[:], in_=wtmax[:], axis=AX.X, op=ALU.max)
            mxs = colp.tile([P, KC], F32, tag="mxs", name="mxs")
            nc.vector.scalar_tensor_tensor(
                out=mxs[:], in0=mxc[:], scalar=-1.0, in1=sc4[:],
                op0=ALU.mult, op1=ALU.mult,
            )
            # Per-tile exp (+row-sum accumulator) so batch 0 finishes its four
            # chunks quickly and stage 5 can start while later batches exp.
            ssum = colp.tile([P, KC, B_LOC], F32, tag="ssum", name="ssum")
            rs = colp.tile([P, KC, B_LOC], F32, tag="rs", name="rs")
            for b in range(B_LOC):
                for kc in range(KC):
                    nc.scalar.activation(
                        out=wt_all[kc][:, b, :], in_=wt_all[kc][:, b, :], func=AF.Exp,
                        bias=mxs[:, kc : kc + 1], scale=sc4[:, kc : kc + 1],
                        accum_out=ssum[:, kc, b : b + 1],
                    )
                    nc.vector.reciprocal(rs[:, kc, b : b + 1], ssum[:, kc, b : b + 1])
                    nc.vector.tensor_scalar_mul(
                        wt_all[kc][:, b, :], wt_all[kc][:, b, :], rs[:, kc, b : b + 1]
                    )

            # ---------------- Stage 5: out = w @ l, BN5 + ELU ----------------
            az5 = accp.tile([P, KC, B_LOC], F32, tag="az", name="az5")
            azq5 = accp.tile([P, KC], F32, tag="azq", name="azq5")
            es_s5 = ExitStack()
            if True:
                outp = es_s5.enter_context(tc.tile_pool(name="out", bufs=KC))
                out_all = [outp.tile([P, B_LOC, L], BF16, tag="out", name="out") for _ in range(KC)]
                for mc in range(KC):
                    for b in range(B_LOC):
                        ps = psum.tile([P, L], F32, tag="ps", name="ps")
                        for kc in range(KC):
                            nc.tensor.matmul(
                                ps[:],
                                lstd_sb[b, kc][:, mc * P : (mc + 1) * P],
                                wt_all[kc][:, b, :],
                                start=(kc == 0),
                                stop=(kc == KC - 1),
                            )
                        nc.vector.tensor_scalar(
                            out=out_all[mc][:, b, :], in0=ps[:],
                            scalar1=1.0, scalar2=0.0, op0=ALU.mult, op1=ALU.add,
                            accum_out=az5[:, mc, b : b + 1],
                        )
                    sumsq_chunk(out_all[mc], azq5, mc)

                pack5 = make_pack(az5, azq5, "pack5")
                g5 = allreduce(pack5, KC * 2)
                sc5, bi5 = affines(g5, _G5, _B5)
                # Tail: one whole-chunk ELU (the affine is constant across
                # batches) then one DMA per chunk.
                for mc in range(KC):
                    zt = out_all[mc][:]
                    e = elup.tile([P, B_LOC, L], BF16, tag="elu_e5", name="elu_e5", bufs=2)
                    y = elup.tile([P, B_LOC, L], BF16, tag="elu_y5", name="elu_y5", bufs=2)
                    sc = sc5[:, mc : mc + 1]
                    bi = bi5[:, mc : mc + 1]
                    nc.scalar.activation(out=e[:], in_=zt, func=AF.Exp, bias=bi, scale=sc)
                    nc.vector.tensor_scalar(
                        out=y[:], in0=zt, scalar1=sc, scalar2=bi,
                        op0=ALU.mult, op1=ALU.add,
                    )
                    nc.vector.tensor_scalar(
                        out=e[:], in0=e[:], scalar1=1.0, scalar2=1.0,
                        op0=ALU.min, op1=ALU.subtract,
                    )
                    nc.vector.tensor_tensor(out=zt, in0=y[:], in1=e[:], op=ALU.max)
                    nc.sync.dma_start(
                        out=out_d[:, mc * P : (mc + 1) * P, :].rearrange(
                            "b p l -> p b l"
                        ),
                        in_=out_all[mc][:],
                    )

                es_s5.close()
                es_l.close()

    _split_waits(nc)
    _CACHE["nc"] = nc
    return nc


def _pack_affine(vecs):
    cols = []
    for v in vecs:
        cols.append(np.ascontiguousarray(np.asarray(v, np.float32).reshape(KC, P).T))
    return np.ascontiguousarray(np.concatenate(cols, axis=1))


def kernel(m1, f, wq, wk, qb, kb, wb, g1, b1, g2, b2, g3, b3, g4, b4, g5, b5):
    BF = ml_dtypes.bfloat16
    m1 = np.asarray(m1, np.float32)
    nc = _build()
    m1t = np.ascontiguousarray(m1.transpose(0, 2, 1)).astype(BF)
    f_h = np.ascontiguousarray(np.asarray(f, np.float32)).astype(BF)
    wq_h = np.ascontiguousarray(np.asarray(wq, np.float32)).astype(BF)
    wk_h = np.ascontiguousarray(np.asarray(wk, np.float32)).astype(BF)
    qbt = np.ascontiguousarray(np.asarray(qb, np.float32).T)
    kbt = np.ascontiguousarray(np.asarray(kb, np.float32).T)
    wbt = np.ascontiguousarray(np.asarray(wb, np.float32).T)
    gpack = _pack_affine([g1, b1, g2, b2, g3, b3, g4, b4, g5, b5])

    shared = {
        "f": f_h, "wq": wq_h, "wk": wk_h,
        "qbt": qbt, "kbt": kbt, "wbt": wbt, "gpack": gpack,
    }
    in_maps = [
        {"m1t": np.ascontiguousarray(m1t[i * B_LOC : (i + 1) * B_LOC]), **shared}
        for i in range(N_CORES)
    ]
    trace = os.environ.get("KERNEL_TRACE") == "1"
    res = run_bass_kernel_spmd(nc, in_maps, list(range(N_CORES)), trace=trace)
    _CACHE["last_results"] = res

    out = np.empty((B, L, E), np.float32)
    for i in range(N_CORES):
        out[i * B_LOC : (i + 1) * B_LOC] = (
            res.results[i]["outt"].astype(np.float32).transpose(0, 2, 1)
        )
    return out
